# revision 1
# baseline (speedup 1.0000x reference)
"""BagOfWords Trainium2 kernel.

Reference computation (per batch b):
    emb    = emb_table[context]                      # (T, D) gather
    logits = emb @ W.T + b                           # (T, V)
    out[t] = (sum_{s<=t} (s+1) * logits[s]) / den[t] # weighted causal cum-avg
    den[t] = (t+1)(t+2)/2

Key identity: the weighted cumsum commutes with the GEMM:
    out[t, v] = (num[t] @ W[v]) / den[t] + b[v]
    num[t, d] = sum_{s<=t} (s+1) * emb[s, d]
so the O(T*V) cumsum collapses onto the tiny (T, D) embedding side.
On device, per 128-token chunk (PE / ACT):
    psum[d, t] = sum_s Xw[s, d] * UT[s, t]          # one matmul per d-chunk
    NT[d, t]   = psum[d, t] + NT_prev[d, last]      # ACT copy w/ bias
with Xw = (s+1)*emb (per-partition scale on ACT) and UT = upper-triangular
ones.  The carry between chunks is just the previous chunk's LAST COLUMN of
NT, consumed as the per-partition bias of the ACT PSUM->SBUF copy -- no
extra matmuls, no cross-engine carry chain (ACT FIFO orders it for free).
NT comes out pre-transposed (d on partitions) = exactly the lhsT layout the
big GEMM wants.  Then out = (NT.T @ W.T) * (1/den[t]) with the normalization
applied as a per-partition scalar in the PSUM->SBUF eviction on DVE, and W.T
staged in SBUF, streamed to HBM.

Sharding (8 cores): 4-way over B x 2-way over V.  Each core gathers 2
batches (2048 rows) but holds only half of W -- that trades +1.6 MB of
gather traffic for -6.1 MB of weight traffic, the DMA-optimal split
(per-core DMA drops 46.8 -> 42.2 MB; the kernel is DMA-bound).

Raw Bass with manual semaphores (one wait per instruction): the walrus build
in this container rejects instructions carrying multiple sem waits, which
rules out the Tile scheduler's multi-wait output.

DMA semaphore discipline: a DMA's 16 per-SDMA-engine sem increments interleave
arbitrarily with other in-flight DMAs on the same semaphore, so a summed
threshold across several outstanding DMAs can fire before a lagging engine
lands its data.  Every concurrently-outstanding DMA group therefore gets its
own semaphore, waited to exactly 16 per iteration.

reps>1 repeats the whole pipeline inside one NEFF (used only for timing: the
benchmark fits a line over reps to cancel the ~50-100 ms axon dispatch
overhead).  Iterations re-gather from the table so every rep computes
identical values; cross-iteration WAR hazards get explicit waits.
"""

import functools
import os
from contextlib import ExitStack

import numpy as np

import concourse.bass as bass
from concourse import mybir
from concourse.bass_utils import run_bass_kernel_spmd

B, T, V, D = 8, 1024, 8000, 384
P = 128
NCORE = 8
NCHUNK = T // P                 # 8 token chunks per batch
KD = D // P                     # 3 contraction chunks
NV = 500                        # vocab tile (one fp32 PSUM bank)
VGRP = 4                        # vocab tiles per store group
NSTAGE = 4                      # output staging buffers
GEMM_BANKS = 6
F32 = mybir.dt.float32
F32R = mybir.dt.float32r

NVG = int(os.environ.get("BOW_NVG", "2"))   # vocab groups (1 or 2)
NB = NVG                        # batches per core (B=8, 8 cores)
V_CORE = V // NVG               # vocab columns per core
BT = NB * T                     # tokens per core
NCHT = NB * NCHUNK              # token chunks per core
NTV = V_CORE // NV              # vocab tiles per core
NGRP = NTV // VGRP              # store column groups
GCOLS = VGRP * NV               # columns per weight/store group

# const-block column layout (single DMA, single sem)
C_POS = 0                       # [128, 8]   pos[p,c] = c*128+p+1
C_UT = C_POS + NCHUNK           # [128, 128] upper-triangular ones (s<=t)
C_IDENC = C_UT + P              # [128, 8]   1/den[c*128+p] column layout
C_DENROW = C_IDENC + NCHUNK     # row 0, [1, 1024] den[t] (bias path only)
C_BIAS = C_DENROW + T           # row 0, [1, V_CORE] (only when has_bias)
CW_NOBIAS = C_IDENC + NCHUNK
CW_BIAS = C_BIAS + V_CORE

# per-iteration semaphore increments
X_IT = NCHT                     # xsem (pos scales)
CT_IT = NCHT * KD               # ctdone / ctsb
GM_IT = NCHT * NTV              # pegemm / evict
GR_IT = GM_IT // VGRP           # store groups


def _build(has_bias: bool, gemm_f32r: bool, reps: int = 1):
    nc = bass.Bass("TRN2", target_bir_lowering=False, debug=False)

    mmdt = F32R if gemm_f32r else F32
    CW = CW_BIAS if has_bias else CW_NOBIAS

    idx_d = nc.dram_tensor("idx", [P, NCHT], mybir.dt.int32, kind="ExternalInput")
    table_d = nc.dram_tensor("table", [V, D], F32, kind="ExternalInput")
    wt_d = nc.dram_tensor("wt", [D, V_CORE], mmdt, kind="ExternalInput")
    consts_d = nc.dram_tensor("consts", [P, CW], F32, kind="ExternalInput")
    out_d = nc.dram_tensor("out", [BT, V_CORE], F32, kind="ExternalOutput")

    with ExitStack() as ctx:
        e = ctx.enter_context
        # SBUF
        idx_sb = e(nc.sbuf_tensor("idx_sb", [P, NCHT], mybir.dt.int32))
        cst = e(nc.sbuf_tensor("cst", [P, CW], F32))
        emb_sb = e(nc.sbuf_tensor("emb_sb", [P, NCHT * D], F32))
        ct_sb = [e(nc.sbuf_tensor(f"ct{k}", [P, BT], mmdt)) for k in range(KD)]
        wt_sb = [e(nc.sbuf_tensor(f"wt{k}", [P, V_CORE], mmdt)) for k in range(KD)]
        ostg = [e(nc.sbuf_tensor(f"ostg{q}", [P, VGRP * NV], F32)) for q in range(NSTAGE)]
        # PSUM (8 banks: 6 gemm + 2 prefix)
        gps = [e(nc.psum_tensor(f"gps{i}", [P, NV], F32)) for i in range(GEMM_BANKS)]
        ctps = [e(nc.psum_tensor(f"ctps{i}", [P, P], F32)) for i in range(2)]
        # sems -- one per concurrently-outstanding DMA group
        csem = e(nc.semaphore("csem"))
        wsem = [[e(nc.semaphore(f"wsem{k}_{g}")) for g in range(NGRP)] for k in range(KD)]
        wsemh = [[e(nc.semaphore(f"wsemh{k}_{g}")) for g in range(NGRP)] for k in range(KD)]
        gidx = e(nc.semaphore("gidx"))
        gsem = [e(nc.semaphore(f"gsem{cc}")) for cc in range(NCHT)]
        osem = [e(nc.semaphore(f"osem{q}")) for q in range(NSTAGE)]
        # engine-progress sems (single-inc, exactly ordered)
        xsem = e(nc.semaphore("xsem"))      # pos-scales done
        ctdone = e(nc.semaphore("ctdone"))  # prefix psum tiles done
        ctsb = e(nc.semaphore("ctsb"))      # NT psum->sbuf copies
        pegemm = e(nc.semaphore("pegemm"))  # gemm psum tiles done
        evict = e(nc.semaphore("evict"))    # gemm evictions on DVE
        blk = e(nc.Block())

        def emb_cc(cc):
            return emb_sb[:, cc * D:(cc + 1) * D]

        pos_ap = lambda c: cst[:, C_POS + c:C_POS + c + 1]
        ut_ap = cst[:, C_UT:C_UT + P]
        idenc_ap = lambda c: cst[:, C_IDENC + c:C_IDENC + c + 1]
        denrow_ap = lambda c: cst[0:1, C_DENROW + c * P:C_DENROW + (c + 1) * P]
        bias_ap = lambda n: cst[0:1, C_BIAS + n * NV:C_BIAS + (n + 1) * NV]

        # half-way gather gate for the weight loads: enough gathers get the
        # DMA engines first, the rest trickle out of Q7 descgen anyway
        G_GATE = min(NCHUNK - 1, NCHT - 1)

        @blk.sync
        def _(sync):
            sync.dma_start(idx_sb[:], idx_d[:]).then_inc(gidx, 16)
            sync.dma_start(cst[:], consts_d[:]).then_inc(csem, 16)
            # column-group-split weight loads staggered between gathers:
            # the gemm sweeps g-outer, so chunk g is needed only g/NGRP of
            # the way through the gemm
            jw = 0
            for g in range(NGRP):
                for k in range(KD):
                    for h in range(2):
                        if jw > 0:
                            sync.wait_ge(gsem[min(jw, NCHT - 1)], 16)
                        jw += 1
                        cols = slice(g * GCOLS + h * GCOLS // 2,
                                     g * GCOLS + (h + 1) * GCOLS // 2)
                        dma = sync.dma_start(wt_sb[k][:, cols],
                                             wt_d[k * P:(k + 1) * P, cols])
                        if h == 1:
                            dma.then_inc(wsem[k][g], 16)
                        else:
                            dma.then_inc(wsemh[k][g], 16)
            # output stores (SP's DGE ring is free once the weights are out)
            for it in range(reps):
                for g in range(NGRP):
                    for mc in range(NCHT):
                        gi = it * GR_IT + g * NCHT + mc
                        sync.wait_ge(evict, (gi + 1) * VGRP)
                        sync.dma_start(
                            out_d[mc * P:(mc + 1) * P, g * GCOLS:(g + 1) * GCOLS],
                            ostg[gi % NSTAGE][:],
                        ).then_inc(osem[gi % NSTAGE], 16)
            for q in range(NSTAGE):
                ngrp_q = (reps * GR_IT - q + NSTAGE - 1) // NSTAGE
                sync.wait_ge(osem[q], 16 * ngrp_q)

        @blk.gpsimd
        def _(gpsimd):
            gpsimd.wait_ge(gidx, 16)
            for it in range(reps):
                for cc in range(NCHT):
                    if it > 0:
                        # WAR: PE must be done reading emb chunk cc of iter it-1
                        gpsimd.wait_ge(ctdone, (it - 1) * CT_IT + (cc + 1) * KD)
                    gpsimd.indirect_dma_start(
                        out=emb_cc(cc),
                        out_offset=None,
                        in_=table_d[:],
                        in_offset=bass.IndirectOffsetOnAxis(ap=idx_sb[:, cc:cc + 1], axis=0),
                    ).then_inc(gsem[cc], 16)

        @blk.scalar
        def _(scalar):
            scalar.wait_ge(csem, 16)
            for it in range(reps):
                # interleave scales with the prefix copies one chunk behind:
                # the copy chain (and with it the gemm) starts after the
                # first gather instead of after the last
                def copies(cc):
                    bt, c = divmod(cc, NCHUNK)
                    for k in range(KD):
                        j = it * CT_IT + cc * KD + k
                        scalar.wait_ge(ctdone, j + 1)
                        if it > 0 and cc == 0 and k == 0:
                            # WAR: gemm of iter it-1 must be done reading ct_sb
                            scalar.wait_ge(pegemm, it * GM_IT)
                        dst = ct_sb[k][:, cc * P:(cc + 1) * P]
                        if c == 0:
                            scalar.copy(dst, ctps[j % 2][:]).then_inc(ctsb, 1)
                        else:
                            carry = ct_sb[k][:, cc * P - 1:cc * P].bitcast(F32)
                            scalar.add(dst, ctps[j % 2][:], carry).then_inc(ctsb, 1)

                for cc in range(NCHT):
                    scalar.wait_ge(gsem[cc], 16 * (it + 1))
                    scalar.mul(emb_cc(cc), emb_cc(cc),
                               pos_ap(cc % NCHUNK)).then_inc(xsem, 1)
                    if cc > 0:
                        copies(cc - 1)
                copies(NCHT - 1)

        @blk.tensor
        def _(tensor):
            tensor.wait_ge(csem, 16)
            for it in range(reps):
                def prefix(cc):
                    # one matmul per (chunk, d-slice); the carry is applied
                    # later by the ACT copy, not here
                    tensor.wait_ge(xsem, it * X_IT + cc + 1)
                    for k in range(KD):
                        j = it * CT_IT + cc * KD + k
                        if j >= 2:
                            tensor.wait_ge(ctsb, j - 1)  # WAR on ctps
                        tensor.matmul(
                            ctps[j % 2][:],
                            lhsT=emb_sb[:, cc * D + k * P: cc * D + (k + 1) * P],
                            rhs=ut_ap,
                            start=True, stop=True).then_inc(ctdone, 1)

                def gemm_block(g, mc):
                    if g == 0:
                        tensor.wait_ge(ctsb, it * CT_IT + KD * (mc + 1))
                    for nin in range(VGRP):
                        n = g * VGRP + nin
                        i = it * GM_IT + (g * NCHT + mc) * VGRP + nin
                        if i >= GEMM_BANKS:
                            tensor.wait_ge(evict, i - GEMM_BANKS + 1)
                        for k in range(KD):
                            if it == 0 and mc == 0 and nin == 0:
                                tensor.wait_ge(wsemh[k][g], 16)
                            if it == 0 and mc == 0 and nin == 2:
                                tensor.wait_ge(wsem[k][g], 16)
                            last = (k == KD - 1) and not has_bias
                            mm = tensor.matmul(
                                gps[i % GEMM_BANKS][:],
                                lhsT=ct_sb[k][:, mc * P:(mc + 1) * P],
                                rhs=wt_sb[k][:, n * NV:(n + 1) * NV],
                                start=(k == 0), stop=last)
                        if has_bias:
                            mm = tensor.matmul(
                                gps[i % GEMM_BANKS][:],
                                lhsT=denrow_ap(mc % NCHUNK),
                                rhs=bias_ap(n),
                                start=False, stop=True)
                        mm.then_inc(pegemm, 1)

                # batch-0 prefix, then the g0 gemm sweep with the next
                # batch's prefix interleaved between blocks (starts the
                # eviction/store pipeline early and leaves no prefix lull
                # between batches); remaining column groups after
                for c in range(NCHUNK):
                    prefix(c)
                for bt in range(NB):
                    for c in range(NCHUNK):
                        gemm_block(0, bt * NCHUNK + c)
                        if bt + 1 < NB:
                            prefix((bt + 1) * NCHUNK + c)
                for g in range(1, NGRP):
                    for mc in range(NCHT):
                        gemm_block(g, mc)

        @blk.vector
        def _(vector):
            for it in range(reps):
                for i0 in range(GM_IT):
                    i = it * GM_IT + i0
                    gi = i // VGRP
                    mc = (i0 // VGRP) % NCHT
                    vector.wait_ge(pegemm, i + 1)
                    if i % VGRP == 0 and gi >= NSTAGE:
                        vector.wait_ge(osem[gi % NSTAGE], 16 * (gi // NSTAGE))
                    vector.tensor_scalar_mul(
                        ostg[gi % NSTAGE][:, (i % VGRP) * NV:(i % VGRP + 1) * NV],
                        gps[i % GEMM_BANKS][:],
                        idenc_ap(mc % NCHUNK)).then_inc(evict, 1)

    return nc


@functools.lru_cache(maxsize=None)
def _get_program(has_bias: bool, gemm_f32r: bool, reps: int = 1):
    return _build(has_bias, gemm_f32r, reps)


@functools.lru_cache(maxsize=None)
def _host_consts(has_bias: bool):
    CW = CW_BIAS if has_bias else CW_NOBIAS
    cst = np.zeros((P, CW), dtype=np.float32)
    t = np.arange(T, dtype=np.float64)
    den = (t + 1.0) * (t + 2.0) / 2.0
    invden = (1.0 / den).astype(np.float32)
    cst[:, C_POS:C_POS + NCHUNK] = (
        np.arange(T, dtype=np.float32) + 1.0).reshape(NCHUNK, P).T
    s = np.arange(P)
    cst[:, C_UT:C_UT + P] = (s[:, None] <= s[None, :]).astype(np.float32)
    cst[:, C_IDENC:C_IDENC + NCHUNK] = invden.reshape(NCHUNK, P).T
    if has_bias:
        cst[0, C_DENROW:C_DENROW + T] = den.astype(np.float32)
    return cst


GEMM_F32R = os.environ.get("BOW_F32R", "1") == "1"  # fp32r: 4x fp32 PE throughput


def make_in_maps(context, emb_table, W, b):
    context = np.asarray(context)
    emb_table = np.ascontiguousarray(np.asarray(emb_table, dtype=np.float32))
    W = np.asarray(W, dtype=np.float32)
    b = np.asarray(b, dtype=np.float32)
    has_bias = bool(np.any(b))

    wt_full = np.ascontiguousarray(W.T)  # (D, V)
    cst0 = _host_consts(has_bias)

    in_maps = []
    for ci in range(NCORE):
        vg, bg = ci % NVG, ci // NVG
        idx = np.concatenate(
            [context[bg * NB + bt].reshape(NCHUNK, P).T for bt in range(NB)],
            axis=1).astype(np.int32)           # [p, cc]
        wt = np.ascontiguousarray(wt_full[:, vg * V_CORE:(vg + 1) * V_CORE])
        cst = cst0
        if has_bias:
            cst = cst0.copy()
            cst[0, C_BIAS:C_BIAS + V_CORE] = b[vg * V_CORE:(vg + 1) * V_CORE]
        in_maps.append({"idx": np.ascontiguousarray(idx), "table": emb_table,
                        "wt": wt, "consts": cst})
    return in_maps, has_bias


def kernel(context, emb_table, W, b):
    in_maps, has_bias = make_in_maps(context, emb_table, W, b)
    nc = _get_program(has_bias, GEMM_F32R)
    try:
        res = run_bass_kernel_spmd(nc, in_maps, list(range(NCORE)))
    except Exception:
        # the axon-tunneled device occasionally reports a transient
        # NRT_EXEC_UNIT_UNRECOVERABLE; one retry reliably clears it
        import time
        time.sleep(2.0)
        res = run_bass_kernel_spmd(nc, in_maps, list(range(NCORE)))
    out = np.empty((B, T, V), dtype=np.float32)
    for ci in range(NCORE):
        vg, bg = ci % NVG, ci // NVG
        o = res.results[ci]["out"]
        for bt in range(NB):
            out[bg * NB + bt, :, vg * V_CORE:(vg + 1) * V_CORE] = \
                o[bt * T:(bt + 1) * T]
    return out



# revision 28
# speedup vs baseline: 1.2510x; 1.2510x over previous
"""BagOfWords Trainium2 kernel.

Reference computation (per batch b):
    emb    = emb_table[context]                      # (T, D) gather
    logits = emb @ W.T + b                           # (T, V)
    out[t] = (sum_{s<=t} (s+1) * logits[s]) / den[t] # weighted causal cum-avg
    den[t] = (t+1)(t+2)/2

Key identity: the weighted cumsum commutes with the GEMM:
    out[t, v] = (num[t] @ W[v]) / den[t] + b[v]
    num[t, d] = sum_{s<=t} (s+1) * emb[s, d]
so the O(T*V) cumsum collapses onto the tiny (T, D) embedding side.
On device, per 128-token chunk (PE / ACT):
    psum[d, t] = sum_s emb[s, d] * UTW_c[s, t]      # one matmul per d-chunk
    NT[d, t]   = psum[d, t] + NT_prev[d, last]      # ACT copy w/ bias
with UTW_c[s, t] = (c*128+s+1) * [s <= t] -- the position weights are folded
into the per-chunk-position upper-triangular matrices (8 of them, bf16), so
there is no separate scale pass and the gather feeds PE directly.  The carry
between chunks is the previous chunk's LAST COLUMN of NT, consumed as the
per-partition bias of the ACT PSUM->SBUF copy (ACT FIFO orders it for free).
NT comes out pre-transposed (d on partitions) = exactly the lhsT layout the
big GEMM wants.  Then out = (NT.T @ W.T) * (1/den[t]) with the normalization
applied as a per-partition scalar in the PSUM->SBUF eviction, distributed
over DVE/Pool/ACT (a single engine cannot keep up with PE), and streamed to
HBM.

Dtypes: the kernel is DMA-bound at f32 (42 MB/core vs the 360 GB/s
DMA-engine aggregate), so traffic is cut with bf16 on every stream whose
precision is not load-bearing: the gathered table (bf16 in HBM), the UTW
prefix operands (also 4x cheaper on PE than fp32r at ap<256), and the OUTPUT
(evictions write bf16, host upcasts).  The NT carry chain stays f32 and the
big GEMM stays fp32r (full-precision W), keeping rel err ~2.7e-3, dominated
by bf16 quantization of emb/pos/out (gate is 2e-2).

Sharding (8 cores): 4-way over B x 2-way over V.  Each core gathers 2
batches but holds only half of W -- the DMA-optimal split.

Per-core DMA ~24 MB (-> ~68 us) vs the PE floor of 82.6 us
(BT/128 * V_CORE * ceil(D/128) * PE_CYCLE for 1-cycle/row dtypes), so the
kernel targets the PE roofline.  Latency killers, in the order the timeline
traces exposed them:
  - prefix PSUM tiles live 4-deep in 2 banks (2 slots each) so the 3
    per-chunk prefix matmuls don't ping-pong PE<->ACT on a 2-buffer WAR;
  - gathers are batched into 5 indirect DMAs (descriptor-gen on Pool is
    ~1us per instruction; 16 singles starved both PE and Pool);
  - the vocab dim is swept in 4 column groups so only 1.5 MB of weights
    gates the first GEMM block, and the k1/k2 slices of that group are
    issued behind the idx load so the chunk-0 gather wins the DMA bus;
  - prefix matmuls are emitted just-in-time (1 block of lead) inside the
    first column-group sweep so PE never parks on a late gather.

Raw Bass with manual semaphores (one wait per instruction): the walrus build
in this container rejects instructions carrying multiple sem waits, which
rules out the Tile scheduler's multi-wait output.

DMA semaphore discipline: a DMA's 16 per-SDMA-engine sem increments interleave
arbitrarily with other in-flight DMAs on the same semaphore, so a summed
threshold across several outstanding DMAs can fire before a lagging engine
lands its data.  Every concurrently-outstanding DMA group therefore gets its
own semaphore, waited to exactly 16 per iteration.

reps>1 repeats the whole pipeline inside one NEFF (used only for timing: the
benchmark fits a line over reps to cancel the ~50-100 ms axon dispatch
overhead).  Iterations re-gather from the table so every rep computes
identical values; cross-iteration WAR hazards get explicit waits.
"""

import functools
import os
from contextlib import ExitStack

import ml_dtypes
import numpy as np

import concourse.bass as bass
from concourse import mybir
from concourse.bass_utils import run_bass_kernel_spmd

B, T, V, D = 8, 1024, 8000, 384
P = 128
NCORE = 8
NCHUNK = T // P                 # 8 token chunks per batch
KD = D // P                     # 3 contraction chunks
NV = 500                        # vocab tile (one fp32 PSUM bank)
VGRP = 2                        # vocab tiles per store/weight group
NSTAGE = 8                      # output staging buffers
GEMM_BANKS = 5
NCTPS = KD                      # one full psum bank per k chain (slot = k);
                                # psum matmul targets must be bank-aligned
F32 = mybir.dt.float32
F32R = mybir.dt.float32r
BF16 = mybir.dt.bfloat16

NVG = int(os.environ.get("BOW_NVG", "2"))   # vocab groups (1 or 2)
NB = NVG                        # batches per core (B=8, 8 cores)
V_CORE = V // NVG               # vocab columns per core
BT = NB * T                     # tokens per core
NCHT = NB * NCHUNK              # token chunks per core
NTV = V_CORE // NV              # vocab tiles per core
NGRP = NTV // VGRP              # store/weight column groups
GCOLS = VGRP * NV               # columns per weight/store group

# gathers: one single-chunk indirect DMA each.  NOTE: batched multi-column
# offset APs simulate correctly in CoreSim but walrus lowers them to a
# DIFFERENT element order on the wire -- compiled results come back wrong.
# Keep one offset column per indirect DMA.
GG = [(c, c + 1) for c in range(NCHT)]
NGG = len(GG)
GRP_OF_CHUNK = {}
for _gi, (_a, _b) in enumerate(GG):
    for _c in range(_a, _b):
        GRP_OF_CHUNK[_c] = _gi

# const-block column layout (single DMA, single sem)
C_IDENC = 0                     # [128, 8]   1/den[c*128+p] column layout
C_DENROW = C_IDENC + NCHUNK     # row 0, [1, 1024] den[t] (bias path only)
C_BIAS = C_DENROW + T           # row 0, [1, V_CORE] (only when has_bias)
CW_NOBIAS = C_IDENC + NCHUNK
CW_BIAS = C_BIAS + V_CORE

# per-iteration semaphore increments
CT_IT = NCHT * KD               # ctdone / ctsb
GM_IT = NCHT * NTV              # pegemm (gemm psum tiles)
GR_IT = GM_IT // VGRP           # store groups

# eviction -> engine map: alternate DVE/ACT (GPSIMD cannot access PSUM --
# walrus birverifier rejects it -- so Pool only runs the gathers)
EVP = ["DA"[i % 2] for i in range(GM_IT)]
EV_CNT = {e: [0] * (GM_IT + 1) for e in "DA"}
for _i, _e in enumerate(EVP):
    for _en in "DA":
        EV_CNT[_en][_i + 1] = EV_CNT[_en][_i] + (_e == _en)
EV_TOT = {e: EV_CNT[e][GM_IT] for e in "DA"}


def _build(has_bias: bool, gemm_f32r: bool, reps: int = 1):
    nc = bass.Bass("TRN2", target_bir_lowering=False, debug=False)

    mmdt = F32R if gemm_f32r else F32
    CW = CW_BIAS if has_bias else CW_NOBIAS

    idx_d = nc.dram_tensor("idx", [P, NCHT], mybir.dt.int32, kind="ExternalInput")
    table_d = nc.dram_tensor("table", [V, D], BF16, kind="ExternalInput")
    wt_d = nc.dram_tensor("wt", [D, V_CORE], mmdt, kind="ExternalInput")
    uth_d = nc.dram_tensor("uth", [P, NCHUNK * P], BF16, kind="ExternalInput")
    consts_d = nc.dram_tensor("consts", [P, CW], F32, kind="ExternalInput")
    out_d = nc.dram_tensor("out", [BT, V_CORE], BF16, kind="ExternalOutput")

    with ExitStack() as ctx:
        e = ctx.enter_context
        # SBUF
        idx_sb = e(nc.sbuf_tensor("idx_sb", [P, NCHT], mybir.dt.int32))
        cst = e(nc.sbuf_tensor("cst", [P, CW], F32))
        uth_sb = e(nc.sbuf_tensor("uth_sb", [P, NCHUNK * P], BF16))
        emb_sb = e(nc.sbuf_tensor("emb_sb", [P, NCHT * D], BF16))
        ct_sb = [e(nc.sbuf_tensor(f"ct{k}", [P, BT], mmdt)) for k in range(KD)]
        wt_sb = [e(nc.sbuf_tensor(f"wt{k}", [P, V_CORE], mmdt)) for k in range(KD)]
        ostg = [e(nc.sbuf_tensor(f"ostg{q}", [P, GCOLS], BF16)) for q in range(NSTAGE)]
        # PSUM (8 banks: 6 gemm + 2 prefix, 2 slots each)
        gps = [e(nc.psum_tensor(f"gps{i}", [P, NV], F32)) for i in range(GEMM_BANKS)]
        ctps_t = [e(nc.psum_tensor(f"ctps{i}", [P, P], F32)) for i in range(NCTPS)]
        ctps = lambda j: ctps_t[j % NCTPS][:]
        # sems -- one per concurrently-outstanding DMA group
        csem = e(nc.semaphore("csem"))
        usem = e(nc.semaphore("usem"))
        wsem = [[e(nc.semaphore(f"wsem{k}_{g}")) for g in range(NGRP)] for k in range(KD)]
        gidx = e(nc.semaphore("gidx"))
        gsem = [e(nc.semaphore(f"gsem{gg}")) for gg in range(NGG)]
        osem = [e(nc.semaphore(f"osem{q}")) for q in range(NSTAGE)]
        # engine-progress sems (single-inc, exactly ordered)
        ctdone = e(nc.semaphore("ctdone"))  # prefix psum tiles done
        ctsb = [e(nc.semaphore(f"ctsb{k}")) for k in range(KD)]  # NT copies
        pegemm = e(nc.semaphore("pegemm"))  # gemm psum tiles done
        evsem = {en: e(nc.semaphore(f"ev{en}")) for en in "DA"}
        blk = e(nc.Block())

        idenc_ap = lambda c: cst[:, C_IDENC + c:C_IDENC + c + 1]
        denrow_ap = lambda c: cst[0:1, C_DENROW + c * P:C_DENROW + (c + 1) * P]
        bias_ap = lambda n: cst[0:1, C_BIAS + n * NV:C_BIAS + (n + 1) * NV]
        utw_ap = lambda c: uth_sb[:, c * P:(c + 1) * P]

        def ev_wait(eng, i):
            # wait for gemm-psum eviction i (global) to be complete
            it, i0 = divmod(i, GM_IT)
            en = EVP[i0]
            eng.wait_ge(evsem[en], it * EV_TOT[en] + EV_CNT[en][i0 + 1])

        @blk.sync
        def _(sync):
            sync.dma_start(idx_sb[:], idx_d[:]).then_inc(gidx, 16)
            sync.dma_start(uth_sb[:], uth_d[:]).then_inc(usem, 16)
            # vocab group 0 weights gate the first GEMM block; k1/k2 are held
            # behind the idx load so the chunk-0 gather slots into the DMA
            # bus between them instead of after them (FIFO by arrival)
            for k in range(KD):
                if k == 1:
                    sync.wait_ge(gidx, 16)
                sync.dma_start(wt_sb[k][:, 0:GCOLS],
                               wt_d[k * P:(k + 1) * P, 0:GCOLS]).then_inc(wsem[k][0], 16)
            sync.dma_start(cst[:], consts_d[:]).then_inc(csem, 16)
            def weights(g):
                for k in range(KD):
                    sync.dma_start(
                        wt_sb[k][:, g * GCOLS:(g + 1) * GCOLS],
                        wt_d[k * P:(k + 1) * P, g * GCOLS:(g + 1) * GCOLS],
                    ).then_inc(wsem[k][g], 16)

            # group-1 weights behind the early gathers; groups 2+ are issued
            # inline in the store stream below (issuing them here would
            # convoy every store behind 12 us of weight transfers)
            sync.wait_ge(gsem[1], 16)
            weights(1)
            # output stores (the globally-last group goes out as per-tile
            # stores so the tail transfer starts right after each eviction)
            slot_cnt = [0] * NSTAGE
            for it in range(reps):
                for g in range(NGRP):
                    for mc in range(NCHT):
                        gi = it * GR_IT + g * NCHT + mc
                        if it == 0 and gi % 16 == 8 and 2 + gi // 16 < NGRP:
                            weights(2 + gi // 16)
                        i0_lo = (g * NCHT + mc) * VGRP
                        slot = gi % NSTAGE
                        rows = out_d[mc * P:(mc + 1) * P, :]
                        if gi == reps * GR_IT - 1:
                            for nin in range(VGRP):
                                i0 = i0_lo + nin
                                en = EVP[i0]
                                sync.wait_ge(evsem[en],
                                             it * EV_TOT[en] + EV_CNT[en][i0 + 1])
                                col = g * GCOLS + nin * NV
                                sync.dma_start(
                                    rows[:, col:col + NV],
                                    ostg[slot][:, nin * NV:(nin + 1) * NV],
                                ).then_inc(osem[slot], 16)
                                slot_cnt[slot] += 1
                            continue
                        for en in "DA":
                            lo = EV_CNT[en][i0_lo]
                            hi = EV_CNT[en][i0_lo + VGRP]
                            if hi > lo:
                                sync.wait_ge(evsem[en], it * EV_TOT[en] + hi)
                        sync.dma_start(
                            rows[:, g * GCOLS:(g + 1) * GCOLS],
                            ostg[slot][:],
                        ).then_inc(osem[slot], 16)
                        slot_cnt[slot] += 1
            for q in range(NSTAGE):
                if slot_cnt[q]:
                    sync.wait_ge(osem[q], 16 * slot_cnt[q])

        def eviction(eng, en, i0, it, state):
            # evict gemm psum tile i0 (of iteration it) on engine en
            if not state[0]:
                eng.wait_ge(csem, 16)           # idenc scalars
                state[0] = True
            i = it * GM_IT + i0
            gi = i // VGRP
            mc = (i0 // VGRP) % NCHT
            if EV_CNT[en][i0] == EV_CNT[en][(i0 // VGRP) * VGRP] and gi >= NSTAGE:
                # first touch of staging slot gi%NSTAGE by this engine
                eng.wait_ge(osem[gi % NSTAGE], 16 * (gi // NSTAGE))
            eng.wait_ge(pegemm, i + 1)
            dst = ostg[gi % NSTAGE][:, (i0 % VGRP) * NV:(i0 % VGRP + 1) * NV]
            src, scl = gps[i % GEMM_BANKS][:], idenc_ap(mc % NCHUNK)
            if en == "A":
                op = eng.mul(dst, src, scl)         # ACT: out = in * scale
            else:
                op = eng.tensor_scalar_mul(dst, src, scl)
            op.then_inc(evsem[en], 1)

        def ev_range(eng, en, lo, hi, it, state):
            for i0 in range(lo, hi):
                if EVP[i0] == en:
                    eviction(eng, en, i0, it, state)

        def copy_ct(eng, k, cc, it, inorder):
            # NT carry-chain copy for d-slice k of chunk cc: psum -> sbuf
            # with the previous chunk's last column as the carry.  The k
            # chains are split over ACT/DVE so a chunk's copies overlap.
            j = it * CT_IT + cc * KD + k
            eng.wait_ge(ctdone, j + 1)
            if it > 0 and cc == 0:
                # WAR: gemm of iter it-1 must be done reading ct_sb
                eng.wait_ge(pegemm, it * GM_IT)
            dst = ct_sb[k][:, cc * P:(cc + 1) * P]
            if cc % NCHUNK == 0:
                if inorder:
                    op = eng.copy(dst, ctps(j))
                else:
                    op = eng.tensor_scalar_add(dst, ctps(j), 0.0)
            else:
                carry = ct_sb[k][:, cc * P - 1:cc * P].bitcast(F32)
                if inorder:
                    # ACT's FIFO orders the carry chain for free
                    op = eng.add(dst, ctps(j), carry)
                else:
                    # DVE's exec queue reorders: sync the chain explicitly
                    eng.wait_ge(ctsb[k], it * NCHT + cc)
                    op = eng.tensor_scalar_add(dst, ctps(j), carry)
            op.then_inc(ctsb[k], 1)

        def chain(eng, ks, en, it, state, inorder):
            # interleave this engine's copy chains with its evictions in
            # PE-production order (prefix mc+1 is emitted before block mc)
            for k in ks:
                copy_ct(eng, k, 0, it, inorder)
            for mc in range(1, NCHT):
                for k in ks:
                    copy_ct(eng, k, mc, it, inorder)
                ev_range(eng, en, (mc - 1) * VGRP, mc * VGRP, it, state)
            ev_range(eng, en, (NCHT - 1) * VGRP, GM_IT, it, state)

        @blk.gpsimd
        def _(gpsimd):
            gpsimd.wait_ge(gidx, 16)
            for it in range(reps):
                for gg, (a, b) in enumerate(GG):
                    if it > 0:
                        # WAR: PE must be done reading these emb chunks (it-1)
                        gpsimd.wait_ge(ctdone, (it - 1) * CT_IT + b * KD)
                    gpsimd.indirect_dma_start(
                        out=emb_sb[:, a * D:b * D],
                        out_offset=None,
                        in_=table_d[:],
                        in_offset=bass.IndirectOffsetOnAxis(ap=idx_sb[:, a:b], axis=0),
                    ).then_inc(gsem[gg], 16)

        @blk.scalar
        def _(scalar):
            st = [False]
            for it in range(reps):
                chain(scalar, (0, 2), "A", it, st, inorder=True)

        @blk.tensor
        def _(tensor):
            tensor.wait_ge(usem, 16)
            if has_bias:
                tensor.wait_ge(csem, 16)
            for it in range(reps):
                def prefix(cc):
                    # one matmul per (chunk, d-slice); the carry is applied
                    # later by the copy chain, not here
                    tensor.wait_ge(gsem[GRP_OF_CHUNK[cc]], 16 * (it + 1))
                    for k in range(KD):
                        j = it * CT_IT + cc * KD + k
                        if j >= NCTPS:
                            # WAR on the ctps slot: wait for the copy that
                            # drained it (slot owner is a different k chain)
                            jj = it * CT_IT + cc * KD + k - NCTPS
                            it2, r = divmod(jj, CT_IT)
                            tensor.wait_ge(ctsb[r % KD], it2 * NCHT + r // KD + 1)
                        tensor.matmul(
                            ctps(j),
                            lhsT=emb_sb[:, cc * D + k * P: cc * D + (k + 1) * P],
                            rhs=utw_ap(cc % NCHUNK),
                            start=True, stop=True).then_inc(ctdone, 1)

                def gemm_block(g, mc):
                    if g == 0:
                        for k in range(KD):
                            tensor.wait_ge(ctsb[k], it * NCHT + mc + 1)
                    for nin in range(VGRP):
                        n = g * VGRP + nin
                        i = it * GM_IT + (g * NCHT + mc) * VGRP + nin
                        if i >= GEMM_BANKS:
                            ev_wait(tensor, i - GEMM_BANKS)
                        for k in range(KD):
                            if it == 0 and mc == 0 and nin == 0:
                                tensor.wait_ge(wsem[k][g], 16)
                            last = (k == KD - 1) and not has_bias
                            mm = tensor.matmul(
                                gps[i % GEMM_BANKS][:],
                                lhsT=ct_sb[k][:, mc * P:(mc + 1) * P],
                                rhs=wt_sb[k][:, n * NV:(n + 1) * NV],
                                start=(k == 0), stop=last)
                        if has_bias:
                            mm = tensor.matmul(
                                gps[i % GEMM_BANKS][:],
                                lhsT=denrow_ap(mc % NCHUNK),
                                rhs=bias_ap(n),
                                start=False, stop=True)
                        mm.then_inc(pegemm, 1)

                # group-0 sweep with just-in-time prefixes (one block of
                # lead); remaining column groups after
                prefix(0)
                gemm_block(0, 0)
                prefix(1)
                for mc in range(1, NCHT):
                    if mc + 1 < NCHT:
                        prefix(mc + 1)
                    gemm_block(0, mc)
                for g in range(1, NGRP):
                    for mc in range(NCHT):
                        gemm_block(g, mc)

        @blk.vector
        def _(vector):
            st = [False]
            for it in range(reps):
                chain(vector, (1,), "D", it, st, inorder=False)

    return nc


@functools.lru_cache(maxsize=None)
def _get_program(has_bias: bool, gemm_f32r: bool, reps: int = 1):
    return _build(has_bias, gemm_f32r, reps)


@functools.lru_cache(maxsize=None)
def _host_consts(has_bias: bool):
    CW = CW_BIAS if has_bias else CW_NOBIAS
    cst = np.zeros((P, CW), dtype=np.float32)
    t = np.arange(T, dtype=np.float64)
    den = (t + 1.0) * (t + 2.0) / 2.0
    invden = (1.0 / den).astype(np.float32)
    cst[:, C_IDENC:C_IDENC + NCHUNK] = invden.reshape(NCHUNK, P).T
    if has_bias:
        cst[0, C_DENROW:C_DENROW + T] = den.astype(np.float32)
    return cst


@functools.lru_cache(maxsize=None)
def _host_uth():
    # UTW_c[s, t] = (c*128 + s + 1) * [s <= t], bf16, c-major blocks
    s = np.arange(P)
    ut = (s[:, None] <= s[None, :]).astype(np.float32)
    uth = np.empty((P, NCHUNK * P), dtype=ml_dtypes.bfloat16)
    for c in range(NCHUNK):
        w = (c * P + s + 1).astype(np.float32)
        uth[:, c * P:(c + 1) * P] = (w[:, None] * ut).astype(ml_dtypes.bfloat16)
    return uth


GEMM_F32R = os.environ.get("BOW_F32R", "1") == "1"  # fp32r: 4x fp32 PE throughput


def make_in_maps(context, emb_table, W, b):
    context = np.asarray(context)
    emb_table = np.ascontiguousarray(
        np.asarray(emb_table, dtype=np.float32).astype(ml_dtypes.bfloat16))
    W = np.asarray(W, dtype=np.float32)
    b = np.asarray(b, dtype=np.float32)
    has_bias = bool(np.any(b))

    wt_full = np.ascontiguousarray(W.T)  # (D, V)
    cst0 = _host_consts(has_bias)
    uth = _host_uth()

    in_maps = []
    for ci in range(NCORE):
        vg, bg = ci % NVG, ci // NVG
        idx = np.concatenate(
            [context[bg * NB + bt].reshape(NCHUNK, P).T for bt in range(NB)],
            axis=1).astype(np.int32)           # [p, cc]
        wt = np.ascontiguousarray(wt_full[:, vg * V_CORE:(vg + 1) * V_CORE])
        cst = cst0
        if has_bias:
            cst = cst0.copy()
            cst[0, C_BIAS:C_BIAS + V_CORE] = b[vg * V_CORE:(vg + 1) * V_CORE]
        in_maps.append({"idx": np.ascontiguousarray(idx), "table": emb_table,
                        "wt": wt, "uth": uth, "consts": cst})
    return in_maps, has_bias


def kernel(context, emb_table, W, b):
    in_maps, has_bias = make_in_maps(context, emb_table, W, b)
    nc = _get_program(has_bias, GEMM_F32R)
    try:
        res = run_bass_kernel_spmd(nc, in_maps, list(range(NCORE)))
    except Exception:
        # the axon-tunneled device occasionally reports a transient
        # NRT_EXEC_UNIT_UNRECOVERABLE; one retry reliably clears it
        import time
        time.sleep(2.0)
        res = run_bass_kernel_spmd(nc, in_maps, list(range(NCORE)))
    out = np.empty((B, T, V), dtype=np.float32)
    for ci in range(NCORE):
        vg, bg = ci % NVG, ci // NVG
        o = np.asarray(res.results[ci]["out"]).astype(np.float32)
        for bt in range(NB):
            out[bg * NB + bt, :, vg * V_CORE:(vg + 1) * V_CORE] = \
                o[bt * T:(bt + 1) * T]
    return out


# revision 33
# speedup vs baseline: 1.2708x; 1.0158x over previous
"""BagOfWords Trainium2 kernel.

Reference computation (per batch b):
    emb    = emb_table[context]                      # (T, D) gather
    logits = emb @ W.T + b                           # (T, V)
    out[t] = (sum_{s<=t} (s+1) * logits[s]) / den[t] # weighted causal cum-avg
    den[t] = (t+1)(t+2)/2

Key identity: the weighted cumsum commutes with the GEMM:
    out[t, v] = (num[t] @ W[v]) / den[t] + b[v]
    num[t, d] = sum_{s<=t} (s+1) * emb[s, d]
so the O(T*V) cumsum collapses onto the tiny (T, D) embedding side.
On device, per 128-token chunk (PE / ACT):
    psum[d, t] = sum_s emb[s, d] * UTW_c[s, t]      # one matmul per d-chunk
    NT[d, t]   = psum[d, t] + NT_prev[d, last]      # ACT copy w/ bias
with UTW_c[s, t] = (c*128+s+1) * [s <= t] -- the position weights are folded
into the per-chunk-position upper-triangular matrices (8 of them, bf16), so
there is no separate scale pass and the gather feeds PE directly.  The carry
between chunks is the previous chunk's LAST COLUMN of NT, consumed as the
per-partition bias of the ACT PSUM->SBUF copy (ACT FIFO orders it for free).
NT comes out pre-transposed (d on partitions) = exactly the lhsT layout the
big GEMM wants.  Then out = (NT.T @ W.T) * (1/den[t]) with the normalization
applied as a per-partition scalar in the PSUM->SBUF eviction, distributed
over DVE/Pool/ACT (a single engine cannot keep up with PE), and streamed to
HBM.

Dtypes: the kernel is DMA-bound at f32 (42 MB/core vs the 360 GB/s
DMA-engine aggregate), so traffic is cut with bf16 on every stream whose
precision is not load-bearing: the gathered table (bf16 in HBM), the UTW
prefix operands (also 4x cheaper on PE than fp32r at ap<256), and the OUTPUT
(evictions write bf16, host upcasts).  The NT carry chain stays f32 and the
big GEMM stays fp32r (full-precision W), keeping rel err ~2.7e-3, dominated
by bf16 quantization of emb/pos/out (gate is 2e-2).

Sharding (8 cores): 4-way over B x 2-way over V.  Each core gathers 2
batches but holds only half of W -- the DMA-optimal split.

Per-core DMA ~24 MB (-> ~68 us) vs the PE floor of 82.6 us
(BT/128 * V_CORE * ceil(D/128) * PE_CYCLE for 1-cycle/row dtypes), so the
kernel targets the PE roofline.  Latency killers, in the order the timeline
traces exposed them:
  - prefix PSUM tiles live 4-deep in 2 banks (2 slots each) so the 3
    per-chunk prefix matmuls don't ping-pong PE<->ACT on a 2-buffer WAR;
  - gathers are batched into 5 indirect DMAs (descriptor-gen on Pool is
    ~1us per instruction; 16 singles starved both PE and Pool);
  - the vocab dim is swept in 4 column groups so only 1.5 MB of weights
    gates the first GEMM block, and the k1/k2 slices of that group are
    issued behind the idx load so the chunk-0 gather wins the DMA bus;
  - prefix matmuls are emitted just-in-time (1 block of lead) inside the
    first column-group sweep so PE never parks on a late gather.

Raw Bass with manual semaphores (one wait per instruction): the walrus build
in this container rejects instructions carrying multiple sem waits, which
rules out the Tile scheduler's multi-wait output.

DMA semaphore discipline: a DMA's 16 per-SDMA-engine sem increments interleave
arbitrarily with other in-flight DMAs on the same semaphore, so a summed
threshold across several outstanding DMAs can fire before a lagging engine
lands its data.  Every concurrently-outstanding DMA group therefore gets its
own semaphore, waited to exactly 16 per iteration.

reps>1 repeats the whole pipeline inside one NEFF (used only for timing: the
benchmark fits a line over reps to cancel the ~50-100 ms axon dispatch
overhead).  Iterations re-gather from the table so every rep computes
identical values; cross-iteration WAR hazards get explicit waits.
"""

import functools
import os
from contextlib import ExitStack

import ml_dtypes
import numpy as np

import concourse.bass as bass
from concourse import mybir
from concourse.bass_utils import run_bass_kernel_spmd

B, T, V, D = 8, 1024, 8000, 384
P = 128
NCORE = 8
NCHUNK = T // P                 # 8 token chunks per batch
KD = D // P                     # 3 contraction chunks
NV = 500                        # vocab tile (one fp32 PSUM bank)
VGRP = 2                        # vocab tiles per store/weight group
NSTAGE = 8                      # output staging buffers
GEMM_BANKS = 5
NCTPS = KD                      # one full psum bank per k chain (slot = k);
                                # psum matmul targets must be bank-aligned
F32 = mybir.dt.float32
F32R = mybir.dt.float32r
BF16 = mybir.dt.bfloat16

NVG = int(os.environ.get("BOW_NVG", "2"))   # vocab groups (1 or 2)
NB = NVG                        # batches per core (B=8, 8 cores)
V_CORE = V // NVG               # vocab columns per core
BT = NB * T                     # tokens per core
NCHT = NB * NCHUNK              # token chunks per core
NTV = V_CORE // NV              # vocab tiles per core
NGRP = NTV // VGRP              # store/weight column groups
GCOLS = VGRP * NV               # columns per weight/store group

# gathers: one single-chunk indirect DMA each.  NOTE: batched multi-column
# offset APs simulate correctly in CoreSim but walrus lowers them to a
# DIFFERENT element order on the wire -- compiled results come back wrong.
# Keep one offset column per indirect DMA.
GG = [(c, c + 1) for c in range(NCHT)]
NGG = len(GG)
GRP_OF_CHUNK = {}
for _gi, (_a, _b) in enumerate(GG):
    for _c in range(_a, _b):
        GRP_OF_CHUNK[_c] = _gi

# const-block column layout (single DMA, single sem)
C_IDENC = 0                     # [128, 8]   1/den[c*128+p] column layout
C_DENROW = C_IDENC + NCHUNK     # row 0, [1, 1024] den[t] (bias path only)
C_BIAS = C_DENROW + T           # row 0, [1, V_CORE] (only when has_bias)
CW_NOBIAS = C_IDENC + NCHUNK
CW_BIAS = C_BIAS + V_CORE

# per-iteration semaphore increments
CT_IT = NCHT * KD               # ctdone / ctsb
GM_IT = NCHT * NTV              # pegemm (gemm psum tiles)
GR_IT = GM_IT // VGRP           # store groups

# eviction -> engine map: alternate DVE/ACT (GPSIMD cannot access PSUM --
# walrus birverifier rejects it -- so Pool only runs the gathers)
EVP = ["DA"[i % 2] for i in range(GM_IT)]
EV_CNT = {e: [0] * (GM_IT + 1) for e in "DA"}
for _i, _e in enumerate(EVP):
    for _en in "DA":
        EV_CNT[_en][_i + 1] = EV_CNT[_en][_i] + (_e == _en)
EV_TOT = {e: EV_CNT[e][GM_IT] for e in "DA"}


def _build(has_bias: bool, gemm_f32r: bool, reps: int = 1):
    nc = bass.Bass("TRN2", target_bir_lowering=False, debug=False)

    mmdt = BF16                 # GEMM operands: W and the NT chain in bf16
    CW = CW_BIAS if has_bias else CW_NOBIAS

    idx_d = nc.dram_tensor("idx", [P, NCHT], mybir.dt.int32, kind="ExternalInput")
    table_d = nc.dram_tensor("table", [V, D], BF16, kind="ExternalInput")
    wt_d = nc.dram_tensor("wt", [D, V_CORE], mmdt, kind="ExternalInput")
    uth_d = nc.dram_tensor("uth", [P, NCHUNK * P], BF16, kind="ExternalInput")
    consts_d = nc.dram_tensor("consts", [P, CW], F32, kind="ExternalInput")
    out_d = nc.dram_tensor("out", [BT, V_CORE], BF16, kind="ExternalOutput")

    with ExitStack() as ctx:
        e = ctx.enter_context
        # SBUF
        idx_sb = e(nc.sbuf_tensor("idx_sb", [P, NCHT], mybir.dt.int32))
        cst = e(nc.sbuf_tensor("cst", [P, CW], F32))
        uth_sb = e(nc.sbuf_tensor("uth_sb", [P, NCHUNK * P], BF16))
        emb_sb = e(nc.sbuf_tensor("emb_sb", [P, NCHT * D], BF16))
        ct_sb = [e(nc.sbuf_tensor(f"ct{k}", [P, BT], mmdt)) for k in range(KD)]
        wt_sb = [e(nc.sbuf_tensor(f"wt{k}", [P, V_CORE], mmdt)) for k in range(KD)]
        ostg = [e(nc.sbuf_tensor(f"ostg{q}", [P, GCOLS], BF16)) for q in range(NSTAGE)]
        carry_sb = e(nc.sbuf_tensor("carry_sb", [P, KD], F32))
        # PSUM (8 banks: 6 gemm + 2 prefix, 2 slots each)
        gps = [e(nc.psum_tensor(f"gps{i}", [P, NV], F32)) for i in range(GEMM_BANKS)]
        ctps_t = [e(nc.psum_tensor(f"ctps{i}", [P, P], F32)) for i in range(NCTPS)]
        ctps = lambda j: ctps_t[j % NCTPS][:]
        # sems -- one per concurrently-outstanding DMA group
        csem = e(nc.semaphore("csem"))
        usem = e(nc.semaphore("usem"))
        wsem = [[e(nc.semaphore(f"wsem{k}_{g}")) for g in range(NGRP)] for k in range(KD)]
        gidx = e(nc.semaphore("gidx"))
        gsem = [e(nc.semaphore(f"gsem{gg}")) for gg in range(NGG)]
        osem = [e(nc.semaphore(f"osem{q}")) for q in range(NSTAGE)]
        # engine-progress sems (single-inc, exactly ordered)
        ctdone = e(nc.semaphore("ctdone"))  # prefix psum tiles done
        ctsb = [e(nc.semaphore(f"ctsb{k}")) for k in range(KD)]  # NT copies
        pegemm = e(nc.semaphore("pegemm"))  # gemm psum tiles done
        evsem = {en: e(nc.semaphore(f"ev{en}")) for en in "DA"}
        blk = e(nc.Block())

        idenc_ap = lambda c: cst[:, C_IDENC + c:C_IDENC + c + 1]
        denrow_ap = lambda c: cst[0:1, C_DENROW + c * P:C_DENROW + (c + 1) * P]
        bias_ap = lambda n: cst[0:1, C_BIAS + n * NV:C_BIAS + (n + 1) * NV]
        utw_ap = lambda c: uth_sb[:, c * P:(c + 1) * P]

        def ev_wait(eng, i):
            # wait for gemm-psum eviction i (global) to be complete
            it, i0 = divmod(i, GM_IT)
            en = EVP[i0]
            eng.wait_ge(evsem[en], it * EV_TOT[en] + EV_CNT[en][i0 + 1])

        @blk.sync
        def _(sync):
            sync.dma_start(idx_sb[:], idx_d[:]).then_inc(gidx, 16)
            sync.dma_start(uth_sb[:], uth_d[:]).then_inc(usem, 16)
            # vocab group 0 weights gate the first GEMM block; k1/k2 are held
            # behind the idx load so the chunk-0 gather slots into the DMA
            # bus between them instead of after them (FIFO by arrival)
            for k in range(KD):
                if k == 1:
                    sync.wait_ge(gidx, 16)
                sync.dma_start(wt_sb[k][:, 0:GCOLS],
                               wt_d[k * P:(k + 1) * P, 0:GCOLS]).then_inc(wsem[k][0], 16)
            sync.dma_start(cst[:], consts_d[:]).then_inc(csem, 16)
            # remaining vocab groups: the mc-major gemm sweep touches all of
            # them within its first 4 blocks, and bf16 keeps the whole load
            # at ~8.5us of bus time
            for g in range(1, NGRP):
                for k in range(KD):
                    sync.dma_start(
                        wt_sb[k][:, g * GCOLS:(g + 1) * GCOLS],
                        wt_d[k * P:(k + 1) * P, g * GCOLS:(g + 1) * GCOLS],
                    ).then_inc(wsem[k][g], 16)
            # output stores (the globally-last group goes out as per-tile
            # stores so the tail transfer starts right after each eviction)
            slot_cnt = [0] * NSTAGE
            for it in range(reps):
                for mc in range(NCHT):
                    for g in range(NGRP):
                        gi = it * GR_IT + mc * NGRP + g
                        i0_lo = (mc * NGRP + g) * VGRP
                        slot = gi % NSTAGE
                        rows = out_d[mc * P:(mc + 1) * P, :]
                        if gi == reps * GR_IT - 1:
                            for nin in range(VGRP):
                                i0 = i0_lo + nin
                                en = EVP[i0]
                                sync.wait_ge(evsem[en],
                                             it * EV_TOT[en] + EV_CNT[en][i0 + 1])
                                col = g * GCOLS + nin * NV
                                sync.dma_start(
                                    rows[:, col:col + NV],
                                    ostg[slot][:, nin * NV:(nin + 1) * NV],
                                ).then_inc(osem[slot], 16)
                                slot_cnt[slot] += 1
                            continue
                        for en in "DA":
                            lo = EV_CNT[en][i0_lo]
                            hi = EV_CNT[en][i0_lo + VGRP]
                            if hi > lo:
                                sync.wait_ge(evsem[en], it * EV_TOT[en] + hi)
                        sync.dma_start(
                            rows[:, g * GCOLS:(g + 1) * GCOLS],
                            ostg[slot][:],
                        ).then_inc(osem[slot], 16)
                        slot_cnt[slot] += 1
            for q in range(NSTAGE):
                if slot_cnt[q]:
                    sync.wait_ge(osem[q], 16 * slot_cnt[q])

        def eviction(eng, en, i0, it, state):
            # evict gemm psum tile i0 (of iteration it) on engine en
            if not state[0]:
                eng.wait_ge(csem, 16)           # idenc scalars
                state[0] = True
            i = it * GM_IT + i0
            gi = i // VGRP
            mc = i0 // (VGRP * NGRP)
            if EV_CNT[en][i0] == EV_CNT[en][(i0 // VGRP) * VGRP] and gi >= NSTAGE:
                # first touch of staging slot gi%NSTAGE by this engine
                eng.wait_ge(osem[gi % NSTAGE], 16 * (gi // NSTAGE))
            eng.wait_ge(pegemm, i + 1)
            dst = ostg[gi % NSTAGE][:, (i0 % VGRP) * NV:(i0 % VGRP + 1) * NV]
            src, scl = gps[i % GEMM_BANKS][:], idenc_ap(mc % NCHUNK)
            if en == "A":
                op = eng.mul(dst, src, scl)         # ACT: out = in * scale
            else:
                op = eng.tensor_scalar_mul(dst, src, scl)
            op.then_inc(evsem[en], 1)

        def ev_range(eng, en, lo, hi, it, state):
            for i0 in range(lo, hi):
                if EVP[i0] == en:
                    eviction(eng, en, i0, it, state)

        def copy_ct(eng, k, cc, it, inorder):
            # NT carry-chain step for d-slice k of chunk cc: (1) the chunk
            # copy psum+carry -> ct_sb (bf16), (2) a tiny f32 carry update
            # carry_sb[:,k] (+)= psum[:,last] so the running total never
            # loses precision to bf16.  Each op bumps ctsb[k] (2 per chunk).
            # The k chains are split over ACT/DVE so a chunk's copies
            # overlap; ACT's FIFO orders its chains for free, DVE's exec
            # queue reorders so its chain is sem-synced explicitly.
            j = it * CT_IT + cc * KD + k
            base = it * 2 * NCHT
            eng.wait_ge(ctdone, j + 1)
            if it > 0 and cc == 0:
                # WAR: gemm of iter it-1 must be done reading ct_sb
                eng.wait_ge(pegemm, it * GM_IT)
            dst = ct_sb[k][:, cc * P:(cc + 1) * P]
            carry = carry_sb[:, k:k + 1]
            last_col = ctps(j)[:, P - 1:P]
            if cc % NCHUNK == 0:
                if inorder:
                    op = eng.copy(dst, ctps(j))
                else:
                    op = eng.tensor_scalar_add(dst, ctps(j), 0.0)
                op.then_inc(ctsb[k], 1)
                if not inorder:
                    eng.wait_ge(ctsb[k], base + 2 * cc + 1)
                if inorder:
                    up = eng.copy(carry, last_col)
                else:
                    up = eng.tensor_scalar_add(carry, last_col, 0.0)
            else:
                if not inorder:
                    eng.wait_ge(ctsb[k], base + 2 * cc)
                if inorder:
                    op = eng.add(dst, ctps(j), carry)
                else:
                    op = eng.tensor_scalar_add(dst, ctps(j), carry)
                op.then_inc(ctsb[k], 1)
                if not inorder:
                    eng.wait_ge(ctsb[k], base + 2 * cc + 1)
                if inorder:
                    up = eng.add(carry, last_col, carry)
                else:
                    up = eng.tensor_scalar_add(carry, last_col, carry)
            up.then_inc(ctsb[k], 1)

        def chain(eng, ks, en, it, state, inorder):
            # interleave this engine's copy chains with its evictions in
            # PE-production order (prefix mc+2 is emitted after block mc)
            mcw = VGRP * NGRP           # tiles per chunk-step
            for cc in (0, 1):
                for k in ks:
                    copy_ct(eng, k, cc, it, inorder)
            for mc in range(NCHT):
                ev_range(eng, en, mc * mcw, (mc + 1) * mcw, it, state)
                if mc + 2 < NCHT:
                    for k in ks:
                        copy_ct(eng, k, mc + 2, it, inorder)

        @blk.gpsimd
        def _(gpsimd):
            gpsimd.wait_ge(gidx, 16)
            for it in range(reps):
                for gg, (a, b) in enumerate(GG):
                    if it == 0 and a == 4:
                        # chunks 4+ aren't needed until deep into the sweep;
                        # keep their transfers out of the DMA-bus queue ahead
                        # of the group-3 weights that gate the 4th GEMM block
                        gpsimd.wait_ge(wsem[KD - 1][NGRP - 1], 16)
                    if it > 0:
                        # WAR: PE must be done reading these emb chunks (it-1)
                        gpsimd.wait_ge(ctdone, (it - 1) * CT_IT + b * KD)
                    gpsimd.indirect_dma_start(
                        out=emb_sb[:, a * D:b * D],
                        out_offset=None,
                        in_=table_d[:],
                        in_offset=bass.IndirectOffsetOnAxis(ap=idx_sb[:, a:b], axis=0),
                    ).then_inc(gsem[gg], 16)

        @blk.scalar
        def _(scalar):
            st = [False]
            for it in range(reps):
                chain(scalar, (0, 2), "A", it, st, inorder=True)

        @blk.tensor
        def _(tensor):
            tensor.wait_ge(usem, 16)
            if has_bias:
                tensor.wait_ge(csem, 16)
            for it in range(reps):
                def prefix(cc):
                    # one matmul per (chunk, d-slice); the carry is applied
                    # later by the copy chain, not here
                    tensor.wait_ge(gsem[GRP_OF_CHUNK[cc]], 16 * (it + 1))
                    for k in range(KD):
                        j = it * CT_IT + cc * KD + k
                        if j >= NCTPS:
                            # WAR on the ctps slot: both chain ops of the
                            # previous chunk's same-k slice must be done
                            jj = it * CT_IT + cc * KD + k - NCTPS
                            it2, r = divmod(jj, CT_IT)
                            tensor.wait_ge(ctsb[r % KD],
                                           it2 * 2 * NCHT + 2 * (r // KD) + 2)
                        tensor.matmul(
                            ctps(j),
                            lhsT=emb_sb[:, cc * D + k * P: cc * D + (k + 1) * P],
                            rhs=utw_ap(cc % NCHUNK),
                            start=True, stop=True).then_inc(ctdone, 1)

                def gemm_block(g, mc):
                    if g == 0:
                        for k in range(KD):
                            tensor.wait_ge(ctsb[k], it * 2 * NCHT + 2 * mc + 1)
                    for nin in range(VGRP):
                        n = g * VGRP + nin
                        i = it * GM_IT + (mc * NGRP + g) * VGRP + nin
                        if i >= GEMM_BANKS:
                            ev_wait(tensor, i - GEMM_BANKS)
                        for k in range(KD):
                            if it == 0 and mc == 0 and nin == 0:
                                tensor.wait_ge(wsem[k][g], 16)
                            last = (k == KD - 1) and not has_bias
                            mm = tensor.matmul(
                                gps[i % GEMM_BANKS][:],
                                lhsT=ct_sb[k][:, mc * P:(mc + 1) * P],
                                rhs=wt_sb[k][:, n * NV:(n + 1) * NV],
                                start=(k == 0), stop=last)
                        if has_bias:
                            mm = tensor.matmul(
                                gps[i % GEMM_BANKS][:],
                                lhsT=denrow_ap(mc % NCHUNK),
                                rhs=bias_ap(n),
                                start=False, stop=True)
                        mm.then_inc(pegemm, 1)

                # mc-major sweep: all vocab groups of one token chunk per
                # step, so the prefix/copy chain work amortizes over 4
                # blocks instead of landing on every block
                prefix(0)
                prefix(1)
                for mc in range(NCHT):
                    for g in range(NGRP):
                        gemm_block(g, mc)
                    if mc + 2 < NCHT:
                        prefix(mc + 2)

        @blk.vector
        def _(vector):
            st = [False]
            for it in range(reps):
                chain(vector, (1,), "D", it, st, inorder=False)

    return nc


@functools.lru_cache(maxsize=None)
def _get_program(has_bias: bool, gemm_f32r: bool, reps: int = 1):
    return _build(has_bias, gemm_f32r, reps)


@functools.lru_cache(maxsize=None)
def _host_consts(has_bias: bool):
    CW = CW_BIAS if has_bias else CW_NOBIAS
    cst = np.zeros((P, CW), dtype=np.float32)
    t = np.arange(T, dtype=np.float64)
    den = (t + 1.0) * (t + 2.0) / 2.0
    invden = (1.0 / den).astype(np.float32)
    cst[:, C_IDENC:C_IDENC + NCHUNK] = invden.reshape(NCHUNK, P).T
    if has_bias:
        cst[0, C_DENROW:C_DENROW + T] = den.astype(np.float32)
    return cst


@functools.lru_cache(maxsize=None)
def _host_uth():
    # UTW_c[s, t] = (c*128 + s + 1) * [s <= t], bf16, c-major blocks
    s = np.arange(P)
    ut = (s[:, None] <= s[None, :]).astype(np.float32)
    uth = np.empty((P, NCHUNK * P), dtype=ml_dtypes.bfloat16)
    for c in range(NCHUNK):
        w = (c * P + s + 1).astype(np.float32)
        uth[:, c * P:(c + 1) * P] = (w[:, None] * ut).astype(ml_dtypes.bfloat16)
    return uth


GEMM_F32R = os.environ.get("BOW_F32R", "1") == "1"  # fp32r: 4x fp32 PE throughput


def make_in_maps(context, emb_table, W, b):
    context = np.asarray(context)
    emb_table = np.ascontiguousarray(
        np.asarray(emb_table, dtype=np.float32).astype(ml_dtypes.bfloat16))
    W = np.asarray(W, dtype=np.float32)
    b = np.asarray(b, dtype=np.float32)
    has_bias = bool(np.any(b))

    wt_full = np.ascontiguousarray(W.T)  # (D, V)
    cst0 = _host_consts(has_bias)
    uth = _host_uth()

    in_maps = []
    for ci in range(NCORE):
        vg, bg = ci % NVG, ci // NVG
        idx = np.concatenate(
            [context[bg * NB + bt].reshape(NCHUNK, P).T for bt in range(NB)],
            axis=1).astype(np.int32)           # [p, cc]
        wt = np.ascontiguousarray(
            wt_full[:, vg * V_CORE:(vg + 1) * V_CORE].astype(ml_dtypes.bfloat16))
        cst = cst0
        if has_bias:
            cst = cst0.copy()
            cst[0, C_BIAS:C_BIAS + V_CORE] = b[vg * V_CORE:(vg + 1) * V_CORE]
        in_maps.append({"idx": np.ascontiguousarray(idx), "table": emb_table,
                        "wt": wt, "uth": uth, "consts": cst})
    return in_maps, has_bias


def kernel(context, emb_table, W, b):
    in_maps, has_bias = make_in_maps(context, emb_table, W, b)
    nc = _get_program(has_bias, GEMM_F32R)
    try:
        res = run_bass_kernel_spmd(nc, in_maps, list(range(NCORE)))
    except Exception:
        # the axon-tunneled device occasionally reports a transient
        # NRT_EXEC_UNIT_UNRECOVERABLE; one retry reliably clears it
        import time
        time.sleep(2.0)
        res = run_bass_kernel_spmd(nc, in_maps, list(range(NCORE)))
    out = np.empty((B, T, V), dtype=np.float32)
    for ci in range(NCORE):
        vg, bg = ci % NVG, ci // NVG
        o = np.asarray(res.results[ci]["out"]).astype(np.float32)
        for bt in range(NB):
            out[bg * NB + bt, :, vg * V_CORE:(vg + 1) * V_CORE] = \
                o[bt * T:(bt + 1) * T]
    return out


# revision 52
# speedup vs baseline: 1.3006x; 1.0234x over previous
"""BagOfWords Trainium2 kernel.

Reference computation (per batch b):
    emb    = emb_table[context]                      # (T, D) gather
    logits = emb @ W.T + b                           # (T, V)
    out[t] = (sum_{s<=t} (s+1) * logits[s]) / den[t] # weighted causal cum-avg
    den[t] = (t+1)(t+2)/2

Key identity: the weighted cumsum commutes with the GEMM:
    out[t, v] = (num[t] @ W[v]) / den[t] + b[v]
    num[t, d] = sum_{s<=t} (s+1) * emb[s, d]
so the O(T*V) cumsum collapses onto the tiny (T, D) embedding side.
On device, per 128-token chunk:
    psum[d, t] = sum_s emb[s, d] * UTW_c[s, t]      # one matmul per d-chunk
    NT[d, t]   = psum[d, t] + carry[d]              # copy w/ carry add
with UTW_c[s, t] = (c*128+s+1) * [s <= t] -- the position weights are folded
into the per-chunk-position upper-triangular matrices (8 of them, bf16), so
there is no separate scale pass and the gather feeds PE directly.  The carry
between chunks lives in a tiny f32 sidecar (carry_sb) maintained by a
per-chunk [128,1] update op, so the running total never loses precision to
the bf16 NT store.  NT comes out pre-transposed (d on partitions) = exactly
the lhsT layout the big GEMM wants.  Then out = (NT.T @ W.T) * (1/den[t])
with the normalization applied as a per-partition scalar in the PSUM->SBUF
eviction, alternated between DVE and ACT (GPSIMD cannot touch PSUM, and a
single engine cannot keep up with PE), and streamed to HBM as bf16.

Dtypes: at f32 the kernel is DMA-bound (42 MB/core vs the 360 GB/s
DMA-engine aggregate), so every stream whose precision is not load-bearing
is bf16: the gathered table, the UTW prefix operands, W and the NT chain
(the GEMM runs bf16 at 1 PE-cycle/row, same rate as fp32r), and the OUTPUT
(evictions write bf16, host upcasts).  PSUM accumulation and the carry
sidecar stay f32.  Rel err ~3e-3, dominated by bf16 quantization of
emb/UTW/NT/W/out (gate is 2e-2).

Sharding (8 cores): 4-way over B x 2-way over V.  Each core gathers 2
batches but holds only half of W -- the DMA-optimal split.

Per-core DMA ~21 MB (-> ~60 us) vs the PE floor of 82.6 us
(BT/128 * V_CORE * ceil(D/128) * PE_CYCLE for 1-cycle/row dtypes), so the
kernel targets the PE roofline.  Latency killers, in the order the timeline
traces exposed them:
  - each k chain owns one full prefix PSUM bank (PE matmuls must target
    bank-aligned PSUM) so the per-chunk prefix matmuls never ping-pong on a
    2-buffer WAR; the copy chains are split over ACT (k0,k2) and DVE (k1);
  - the GEMM sweeps chunk-major per BLK_ORDER so the prefix/copy work
    amortizes over NGRP blocks, with the first chunks' last vocab group
    deferred to the sweep tail so its weights stay off the startup bus;
  - gathers, weights, and constants are ordered/gated so the DMA-bus FIFO
    serves each consumer just before PE needs it (see GATE_A/GATE_B);
  - the globally-last store group goes out per-tile so the tail transfer
    starts right after each final eviction.

Raw Bass with manual semaphores (one wait per instruction): the walrus build
in this container rejects instructions carrying multiple sem waits, which
rules out the Tile scheduler's multi-wait output.

DMA semaphore discipline: a DMA's 16 per-SDMA-engine sem increments interleave
arbitrarily with other in-flight DMAs on the same semaphore, so a summed
threshold across several outstanding DMAs can fire before a lagging engine
lands its data.  Every concurrently-outstanding DMA group therefore gets its
own semaphore, waited to exactly 16 per iteration.

reps>1 repeats the whole pipeline inside one NEFF (used only for timing: the
benchmark fits a line over reps to cancel the ~50-100 ms axon dispatch
overhead).  Iterations re-gather from the table so every rep computes
identical values; cross-iteration WAR hazards get explicit waits.
"""

import functools
import os
from contextlib import ExitStack

import ml_dtypes
import numpy as np

import concourse.bass as bass
from concourse import mybir
from concourse.bass_utils import run_bass_kernel_spmd

B, T, V, D = 8, 1024, 8000, 384
P = 128
NCORE = 8
NCHUNK = T // P                 # 8 token chunks per batch
KD = D // P                     # 3 contraction chunks
NV = 500                        # vocab tile (one fp32 PSUM bank)
VGRP = 2                        # vocab tiles per store/weight group
NSTAGE = 8                      # output staging buffers
GEMM_BANKS = 5
NCTPS = KD                      # one full psum bank per k chain (slot = k);
                                # psum matmul targets must be bank-aligned
F32 = mybir.dt.float32
F32R = mybir.dt.float32r
BF16 = mybir.dt.bfloat16

NVG = int(os.environ.get("BOW_NVG", "2"))   # vocab groups (1 or 2)
NB = NVG                        # batches per core (B=8, 8 cores)
V_CORE = V // NVG               # vocab columns per core
BT = NB * T                     # tokens per core
NCHT = NB * NCHUNK              # token chunks per core
NTV = V_CORE // NV              # vocab tiles per core
NGRP = NTV // VGRP              # store/weight column groups
GCOLS = VGRP * NV               # columns per weight/store group

# gathers: one single-chunk indirect DMA each.  NOTE: batched multi-column
# offset APs simulate correctly in CoreSim but walrus lowers them to a
# DIFFERENT element order on the wire -- compiled results come back wrong.
# Keep one offset column per indirect DMA.
GG = [(c, c + 1) for c in range(NCHT)]
NGG = len(GG)
GRP_OF_CHUNK = {}
for _gi, (_a, _b) in enumerate(GG):
    for _c in range(_a, _b):
        GRP_OF_CHUNK[_c] = _gi

# const-block column layout (single DMA, single sem)
C_IDENC = 0                     # [128, 8]   1/den[c*128+p] column layout
C_DENROW = C_IDENC + NCHUNK     # row 0, [1, 1024] den[t] (bias path only)
C_BIAS = C_DENROW + T           # row 0, [1, V_CORE] (only when has_bias)
CW_NOBIAS = C_IDENC + NCHUNK
CW_BIAS = C_BIAS + V_CORE

# per-iteration semaphore increments
CT_IT = NCHT * KD               # ctdone (prefix matmuls)
GM_IT = NCHT * NTV              # pegemm (gemm psum tiles)
GR_IT = GM_IT // VGRP           # store groups

# GEMM block production order: chunk-major (all vocab groups of one token
# chunk per step) so the prefix/copy-chain work amortizes over NGRP blocks
# instead of landing on every block -- EXCEPT the last vocab group of the
# first 3 chunks, which is deferred to the very end of the sweep: the DMA
# bus cannot deliver group 3's weights before the plain chunk-major order
# reaches them, and their token chunks' NT columns stay resident anyway.
_DEFER = [(mc, NGRP - 1) for mc in range(min(int(os.environ.get("BOW_DEFER", "3")), NCHT))]
BLK_ORDER = [(mc, g) for mc in range(NCHT) for g in range(NGRP)
             if (mc, g) not in _DEFER] + _DEFER
TILE_MC = [mc for (mc, g) in BLK_ORDER for _ in range(VGRP)]
FIRST_POS_OF_G = {}
for _pos, (_mc, _g) in enumerate(BLK_ORDER):
    FIRST_POS_OF_G.setdefault(_g, _pos)

# gather-gate positions: the chunk at GATE_A waits for the group-2 weights,
# the chunk at GATE_B for the group-3 weights (DMA-bus priority, see above)
GATE_A = int(os.environ.get("BOW_GATE_A", "3"))
GATE_B = int(os.environ.get("BOW_GATE_B", "5"))

# eviction -> engine map: alternate DVE/ACT (GPSIMD cannot access PSUM --
# walrus birverifier rejects it -- so Pool only runs the gathers)
EVP = [os.environ.get("BOW_EVP", "DA")[i % 2] for i in range(GM_IT)]
EV_CNT = {e: [0] * (GM_IT + 1) for e in "DA"}
for _i, _e in enumerate(EVP):
    for _en in "DA":
        EV_CNT[_en][_i + 1] = EV_CNT[_en][_i] + (_e == _en)
EV_TOT = {e: EV_CNT[e][GM_IT] for e in "DA"}


def _build(has_bias: bool, gemm_f32r: bool, reps: int = 1):
    nc = bass.Bass("TRN2", target_bir_lowering=False, debug=False)

    mmdt = BF16                 # GEMM operands: W and the NT chain in bf16
    CW = CW_BIAS if has_bias else CW_NOBIAS

    idx_d = nc.dram_tensor("idx", [P, NCHT], mybir.dt.int32, kind="ExternalInput")
    table_d = nc.dram_tensor("table", [V, D], BF16, kind="ExternalInput")
    wt_d = nc.dram_tensor("wt", [D, V_CORE], mmdt, kind="ExternalInput")
    uth_d = nc.dram_tensor("uth", [P, NCHUNK * P], BF16, kind="ExternalInput")
    consts_d = nc.dram_tensor("consts", [P, CW], F32, kind="ExternalInput")
    out_d = nc.dram_tensor("out", [BT, V_CORE], BF16, kind="ExternalOutput")

    with ExitStack() as ctx:
        e = ctx.enter_context
        # SBUF
        idx_sb = e(nc.sbuf_tensor("idx_sb", [P, NCHT], mybir.dt.int32))
        cst = e(nc.sbuf_tensor("cst", [P, CW], F32))
        uth_sb = e(nc.sbuf_tensor("uth_sb", [P, NCHUNK * P], BF16))
        emb_sb = e(nc.sbuf_tensor("emb_sb", [P, NCHT * D], BF16))
        ct_sb = [e(nc.sbuf_tensor(f"ct{k}", [P, BT], mmdt)) for k in range(KD)]
        wt_sb = [e(nc.sbuf_tensor(f"wt{k}", [P, V_CORE], mmdt)) for k in range(KD)]
        ostg = [e(nc.sbuf_tensor(f"ostg{q}", [P, GCOLS], BF16)) for q in range(NSTAGE)]
        carry_sb = e(nc.sbuf_tensor("carry_sb", [P, KD], F32))
        # PSUM (8 banks: 6 gemm + 2 prefix, 2 slots each)
        gps = [e(nc.psum_tensor(f"gps{i}", [P, NV], F32)) for i in range(GEMM_BANKS)]
        ctps_t = [e(nc.psum_tensor(f"ctps{i}", [P, P], F32)) for i in range(NCTPS)]
        ctps = lambda j: ctps_t[j % NCTPS][:]
        # sems -- one per concurrently-outstanding DMA group
        csem = e(nc.semaphore("csem"))
        usem = e(nc.semaphore("usem"))
        wsem = [[e(nc.semaphore(f"wsem{k}_{g}")) for g in range(NGRP)] for k in range(KD)]
        gidx = e(nc.semaphore("gidx"))
        gsem = [e(nc.semaphore(f"gsem{gg}")) for gg in range(NGG)]
        osem = [e(nc.semaphore(f"osem{q}")) for q in range(NSTAGE)]
        # engine-progress sems (single-inc, exactly ordered)
        ctdone = e(nc.semaphore("ctdone"))  # prefix psum tiles done
        ctsb = [e(nc.semaphore(f"ctsb{k}")) for k in range(KD)]  # NT copies
        pegemm = e(nc.semaphore("pegemm"))  # gemm psum tiles done
        evsem = {en: e(nc.semaphore(f"ev{en}")) for en in "DA"}
        blk = e(nc.Block())

        idenc_ap = lambda c: cst[:, C_IDENC + c:C_IDENC + c + 1]
        denrow_ap = lambda c: cst[0:1, C_DENROW + c * P:C_DENROW + (c + 1) * P]
        bias_ap = lambda n: cst[0:1, C_BIAS + n * NV:C_BIAS + (n + 1) * NV]
        utw_ap = lambda c: uth_sb[:, c * P:(c + 1) * P]

        def ev_wait(eng, i):
            # wait for gemm-psum eviction i (global) to be complete
            it, i0 = divmod(i, GM_IT)
            en = EVP[i0]
            eng.wait_ge(evsem[en], it * EV_TOT[en] + EV_CNT[en][i0 + 1])

        @blk.sync
        def _(sync):
            sync.dma_start(idx_sb[:], idx_d[:]).then_inc(gidx, 16)
            sync.dma_start(uth_sb[:], uth_d[:]).then_inc(usem, 16)
            # weights in sweep-demand order; group 3 is deferred in
            # BLK_ORDER so its DMAs can sit last.  cst (needed only by the
            # first eviction, ~9us) goes after group 1 to keep its HWDGE
            # descriptor-gen slot off the weight critical path.
            def weights(g):
                for k in range(KD):
                    sync.dma_start(
                        wt_sb[k][:, g * GCOLS:(g + 1) * GCOLS],
                        wt_d[k * P:(k + 1) * P, g * GCOLS:(g + 1) * GCOLS],
                    ).then_inc(wsem[k][g], 16)
            for g in range(NGRP):
                weights(g)
            # output stores (the globally-last group goes out as per-tile
            # stores so the tail transfer starts right after each eviction)
            slot_cnt = [0] * NSTAGE
            for it in range(reps):
                for pos, (mc, g) in enumerate(BLK_ORDER):
                    gi = it * GR_IT + pos
                    i0_lo = pos * VGRP
                    slot = gi % NSTAGE
                    rows = out_d[mc * P:(mc + 1) * P, :]
                    if gi == reps * GR_IT - 1:
                        for nin in range(VGRP):
                            i0 = i0_lo + nin
                            en = EVP[i0]
                            sync.wait_ge(evsem[en],
                                         it * EV_TOT[en] + EV_CNT[en][i0 + 1])
                            col = g * GCOLS + nin * NV
                            sync.dma_start(
                                rows[:, col:col + NV],
                                ostg[slot][:, nin * NV:(nin + 1) * NV],
                            ).then_inc(osem[slot], 16)
                            slot_cnt[slot] += 1
                        continue
                    for en in "DA":
                        lo = EV_CNT[en][i0_lo]
                        hi = EV_CNT[en][i0_lo + VGRP]
                        if hi > lo:
                            sync.wait_ge(evsem[en], it * EV_TOT[en] + hi)
                    sync.dma_start(
                        rows[:, g * GCOLS:(g + 1) * GCOLS],
                        ostg[slot][:],
                    ).then_inc(osem[slot], 16)
                    slot_cnt[slot] += 1
            for q in range(NSTAGE):
                if slot_cnt[q]:
                    sync.wait_ge(osem[q], 16 * slot_cnt[q])

        def eviction(eng, en, i0, it, state):
            # evict gemm psum tile i0 (of iteration it) on engine en
            if not state[0]:
                eng.wait_ge(csem, 16)           # idenc scalars
                state[0] = True
            i = it * GM_IT + i0
            gi = i // VGRP
            mc = TILE_MC[i0]
            if EV_CNT[en][i0] == EV_CNT[en][(i0 // VGRP) * VGRP] and gi >= NSTAGE:
                # first touch of staging slot gi%NSTAGE by this engine
                eng.wait_ge(osem[gi % NSTAGE], 16 * (gi // NSTAGE))
            eng.wait_ge(pegemm, i + 1)
            dst = ostg[gi % NSTAGE][:, (i0 % VGRP) * NV:(i0 % VGRP + 1) * NV]
            src, scl = gps[i % GEMM_BANKS][:], idenc_ap(mc % NCHUNK)
            if en == "A":
                op = eng.mul(dst, src, scl)         # ACT: out = in * scale
            else:
                op = eng.tensor_scalar_mul(dst, src, scl)
            op.then_inc(evsem[en], 1)

        def ev_range(eng, en, lo, hi, it, state):
            for i0 in range(lo, hi):
                if EVP[i0] == en:
                    eviction(eng, en, i0, it, state)

        def copy_ct(eng, k, cc, it, inorder, carry_upd):
            # NT carry-chain step for d-slice k of chunk cc: (1) the chunk
            # copy psum+carry -> ct_sb (bf16), (2) a tiny f32 carry update
            # carry_sb[:,k] (+)= psum[:,last] so the running total never
            # loses precision to bf16.  Each op bumps ctsb[k] (2 per chunk).
            # The k chains are split over ACT/DVE so a chunk's copies
            # overlap; ACT's FIFO orders its chains for free, DVE's exec
            # queue reorders so its chain is sem-synced explicitly.
            j = it * CT_IT + cc * KD + k
            base = it * 2 * NCHT
            fresh = cc % NCHUNK == 0        # new batch: no carry to add
            dst = ct_sb[k][:, cc * P:(cc + 1) * P]
            carry = carry_sb[:, k:k + 1]
            last_col = ctps(j)[:, P - 1:P]
            if not carry_upd:
                eng.wait_ge(ctdone, j + 1)
                if it > 0 and cc == 0:
                    # WAR: gemm of iter it-1 must be done reading ct_sb
                    eng.wait_ge(pegemm, it * GM_IT)
                if fresh:
                    if inorder:
                        op = eng.copy(dst, ctps(j))
                    else:
                        op = eng.tensor_scalar_add(dst, ctps(j), 0.0)
                else:
                    if not inorder:
                        eng.wait_ge(ctsb[k], base + 2 * cc)
                    if inorder:
                        op = eng.add(dst, ctps(j), carry)
                    else:
                        op = eng.tensor_scalar_add(dst, ctps(j), carry)
            else:
                if not inorder:
                    eng.wait_ge(ctsb[k], base + 2 * cc + 1)
                if fresh:
                    if inorder:
                        op = eng.copy(carry, last_col)
                    else:
                        op = eng.tensor_scalar_add(carry, last_col, 0.0)
                else:
                    if inorder:
                        op = eng.add(carry, last_col, carry)
                    else:
                        op = eng.tensor_scalar_add(carry, last_col, carry)
            op.then_inc(ctsb[k], 1)

        def chain(eng, ks, en, it, state, inorder):
            # interleave this engine's copy chains with its evictions in
            # PE-production order (prefix mc+1 lands after block (mc, g=1)).
            # All chunk copies go before all carry updates: the GEMM waits
            # only on the copies, the carries have a full chunk-step of
            # slack.
            def copies(cc):
                for k in ks:
                    copy_ct(eng, k, cc, it, inorder, carry_upd=False)
                for k in ks:
                    copy_ct(eng, k, cc, it, inorder, carry_upd=True)

            copies(0)
            for pos, (mc, g) in enumerate(BLK_ORDER):
                ev_range(eng, en, pos * VGRP, (pos + 1) * VGRP, it, state)
                if g == 0 and mc + 1 < NCHT:
                    copies(mc + 1)

        @blk.gpsimd
        def _(gpsimd):
            # cst rides Pool's SWDGE: its descgen fits entirely inside the
            # idx-load latency, so it costs no HWDGE slot and the evictions'
            # csem wait clears long before the first psum tile lands
            gpsimd.dma_start(cst[:], consts_d[:]).then_inc(csem, 16)
            gpsimd.wait_ge(gidx, 16)
            for it in range(reps):
                for gg, (a, b) in enumerate(GG):
                    if it == 0 and a == 3:
                        # later chunks aren't needed until deep into the
                        # sweep; keep their transfers out of the DMA-bus
                        # queue ahead of the group-2/3 weights that gate
                        # the 3rd/4th GEMM blocks
                        gpsimd.wait_ge(wsem[KD - 1][min(2, NGRP - 1)], 16)
                    if it == 0 and a == 5:
                        gpsimd.wait_ge(wsem[KD - 1][NGRP - 1], 16)
                    if it > 0:
                        # WAR: PE must be done reading these emb chunks (it-1)
                        gpsimd.wait_ge(ctdone, (it - 1) * CT_IT + b * KD)
                    gpsimd.indirect_dma_start(
                        out=emb_sb[:, a * D:b * D],
                        out_offset=None,
                        in_=table_d[:],
                        in_offset=bass.IndirectOffsetOnAxis(ap=idx_sb[:, a:b], axis=0),
                    ).then_inc(gsem[gg], 16)

        @blk.scalar
        def _(scalar):
            st = [False]
            for it in range(reps):
                chain(scalar, (0, 2), "A", it, st, inorder=True)

        @blk.tensor
        def _(tensor):
            tensor.wait_ge(usem, 16)
            if has_bias:
                tensor.wait_ge(csem, 16)
            for it in range(reps):
                def prefix(cc):
                    # one matmul per (chunk, d-slice); the carry is applied
                    # later by the copy chain, not here
                    tensor.wait_ge(gsem[GRP_OF_CHUNK[cc]], 16 * (it + 1))
                    for k in range(KD):
                        j = it * CT_IT + cc * KD + k
                        if j >= NCTPS:
                            # WAR on the ctps slot: both chain ops of the
                            # previous chunk's same-k slice must be done
                            jj = it * CT_IT + cc * KD + k - NCTPS
                            it2, r = divmod(jj, CT_IT)
                            tensor.wait_ge(ctsb[r % KD],
                                           it2 * 2 * NCHT + 2 * (r // KD) + 2)
                        tensor.matmul(
                            ctps(j),
                            lhsT=emb_sb[:, cc * D + k * P: cc * D + (k + 1) * P],
                            rhs=utw_ap(cc % NCHUNK),
                            start=True, stop=True).then_inc(ctdone, 1)

                def gemm_block(pos):
                    mc, g = BLK_ORDER[pos]
                    if g == 0:
                        for k in range(KD):
                            tensor.wait_ge(ctsb[k], it * 2 * NCHT + 2 * mc + 1)
                    for nin in range(VGRP):
                        n = g * VGRP + nin
                        i = it * GM_IT + pos * VGRP + nin
                        if i >= GEMM_BANKS:
                            ev_wait(tensor, i - GEMM_BANKS)
                        for k in range(KD):
                            if it == 0 and pos == FIRST_POS_OF_G[g] and nin == 0:
                                tensor.wait_ge(wsem[k][g], 16)
                            last = (k == KD - 1) and not has_bias
                            mm = tensor.matmul(
                                gps[i % GEMM_BANKS][:],
                                lhsT=ct_sb[k][:, mc * P:(mc + 1) * P],
                                rhs=wt_sb[k][:, n * NV:(n + 1) * NV],
                                start=(k == 0), stop=last)
                        if has_bias:
                            mm = tensor.matmul(
                                gps[i % GEMM_BANKS][:],
                                lhsT=denrow_ap(mc % NCHUNK),
                                rhs=bias_ap(n),
                                start=False, stop=True)
                        mm.then_inc(pegemm, 1)

                # chunk-major sweep per BLK_ORDER; prefixes are emitted
                # just-in-time after each chunk's g==1 block (>=1 chunk-step
                # of lead over their first consumer)
                prefix(0)
                for pos, (mc, g) in enumerate(BLK_ORDER):
                    gemm_block(pos)
                    if g == 0 and mc + 1 < NCHT:
                        prefix(mc + 1)

        @blk.vector
        def _(vector):
            st = [False]
            for it in range(reps):
                chain(vector, (1,), "D", it, st, inorder=False)

    return nc


@functools.lru_cache(maxsize=None)
def _get_program(has_bias: bool, gemm_f32r: bool, reps: int = 1):
    return _build(has_bias, gemm_f32r, reps)


@functools.lru_cache(maxsize=None)
def _host_consts(has_bias: bool):
    CW = CW_BIAS if has_bias else CW_NOBIAS
    cst = np.zeros((P, CW), dtype=np.float32)
    t = np.arange(T, dtype=np.float64)
    den = (t + 1.0) * (t + 2.0) / 2.0
    invden = (1.0 / den).astype(np.float32)
    cst[:, C_IDENC:C_IDENC + NCHUNK] = invden.reshape(NCHUNK, P).T
    if has_bias:
        cst[0, C_DENROW:C_DENROW + T] = den.astype(np.float32)
    return cst


@functools.lru_cache(maxsize=None)
def _host_uth():
    # UTW_c[s, t] = (c*128 + s + 1) * [s <= t], bf16, c-major blocks
    s = np.arange(P)
    ut = (s[:, None] <= s[None, :]).astype(np.float32)
    uth = np.empty((P, NCHUNK * P), dtype=ml_dtypes.bfloat16)
    for c in range(NCHUNK):
        w = (c * P + s + 1).astype(np.float32)
        uth[:, c * P:(c + 1) * P] = (w[:, None] * ut).astype(ml_dtypes.bfloat16)
    return uth


GEMM_F32R = os.environ.get("BOW_F32R", "1") == "1"  # fp32r: 4x fp32 PE throughput


def make_in_maps(context, emb_table, W, b):
    context = np.asarray(context)
    emb_table = np.ascontiguousarray(
        np.asarray(emb_table, dtype=np.float32).astype(ml_dtypes.bfloat16))
    W = np.asarray(W, dtype=np.float32)
    b = np.asarray(b, dtype=np.float32)
    has_bias = bool(np.any(b))

    wt_full = np.ascontiguousarray(W.T)  # (D, V)
    cst0 = _host_consts(has_bias)
    uth = _host_uth()

    in_maps = []
    for ci in range(NCORE):
        vg, bg = ci % NVG, ci // NVG
        idx = np.concatenate(
            [context[bg * NB + bt].reshape(NCHUNK, P).T for bt in range(NB)],
            axis=1).astype(np.int32)           # [p, cc]
        wt = np.ascontiguousarray(
            wt_full[:, vg * V_CORE:(vg + 1) * V_CORE].astype(ml_dtypes.bfloat16))
        cst = cst0
        if has_bias:
            cst = cst0.copy()
            cst[0, C_BIAS:C_BIAS + V_CORE] = b[vg * V_CORE:(vg + 1) * V_CORE]
        in_maps.append({"idx": np.ascontiguousarray(idx), "table": emb_table,
                        "wt": wt, "uth": uth, "consts": cst})
    return in_maps, has_bias


def kernel(context, emb_table, W, b):
    in_maps, has_bias = make_in_maps(context, emb_table, W, b)
    nc = _get_program(has_bias, GEMM_F32R)
    try:
        res = run_bass_kernel_spmd(nc, in_maps, list(range(NCORE)))
    except Exception:
        # the axon-tunneled device occasionally reports a transient
        # NRT_EXEC_UNIT_UNRECOVERABLE; one retry reliably clears it
        import time
        time.sleep(2.0)
        res = run_bass_kernel_spmd(nc, in_maps, list(range(NCORE)))
    out = np.empty((B, T, V), dtype=np.float32)
    for ci in range(NCORE):
        vg, bg = ci % NVG, ci // NVG
        o = np.asarray(res.results[ci]["out"]).astype(np.float32)
        for bt in range(NB):
            out[bg * NB + bt, :, vg * V_CORE:(vg + 1) * V_CORE] = \
                o[bt * T:(bt + 1) * T]
    return out


# revision 64
# speedup vs baseline: 1.3124x; 1.0091x over previous
"""BagOfWords Trainium2 kernel.

Reference computation (per batch b):
    emb    = emb_table[context]                      # (T, D) gather
    logits = emb @ W.T + b                           # (T, V)
    out[t] = (sum_{s<=t} (s+1) * logits[s]) / den[t] # weighted causal cum-avg
    den[t] = (t+1)(t+2)/2

Key identity: the weighted cumsum commutes with the GEMM:
    out[t, v] = (num[t] @ W[v]) / den[t] + b[v]
    num[t, d] = sum_{s<=t} (s+1) * emb[s, d]
so the O(T*V) cumsum collapses onto the tiny (T, D) embedding side.
On device, per 128-token chunk:
    psum[d, t] = sum_s emb[s, d] * UTW_c[s, t]      # one matmul per d-chunk
    NT[d, t]   = psum[d, t] + carry[d]              # copy w/ carry add
with UTW_c[s, t] = (c*128+s+1) * [s <= t] -- the position weights are folded
into the per-chunk-position upper-triangular matrices (8 of them, bf16), so
there is no separate scale pass and the gather feeds PE directly.  The carry
between chunks lives in a tiny f32 sidecar (carry_sb) maintained by a
per-chunk [128,1] update op, so the running total never loses precision to
the bf16 NT store.  NT comes out pre-transposed (d on partitions) = exactly
the lhsT layout the big GEMM wants.  Then out = (NT.T @ W.T) * (1/den[t])
with the normalization applied as a per-partition scalar in the PSUM->SBUF
eviction, alternated between DVE and ACT (GPSIMD cannot touch PSUM, and a
single engine cannot keep up with PE), and streamed to HBM as bf16.

Dtypes: at f32 the kernel is DMA-bound (42 MB/core vs the 360 GB/s
DMA-engine aggregate), so every stream whose precision is not load-bearing
is bf16: the gathered table, the UTW prefix operands, W and the NT chain
(the GEMM runs bf16 at 1 PE-cycle/row, same rate as fp32r), and the OUTPUT
(evictions write bf16, host upcasts).  PSUM accumulation and the carry
sidecar stay f32.  Rel err ~3e-3, dominated by bf16 quantization of
emb/UTW/NT/W/out (gate is 2e-2).

Sharding (8 cores): 4-way over B x 2-way over V.  Each core gathers 2
batches but holds only half of W -- the DMA-optimal split.

Per-core DMA ~21 MB (-> ~60 us) vs the PE floor of 82.6 us
(BT/128 * V_CORE * ceil(D/128) * PE_CYCLE for 1-cycle/row dtypes), so the
kernel targets the PE roofline.  Latency killers, in the order the timeline
traces exposed them:
  - each k chain owns one full prefix PSUM bank (PE matmuls must target
    bank-aligned PSUM) so the per-chunk prefix matmuls never ping-pong on a
    2-buffer WAR; the copy chains are split over ACT (k0,k2) and DVE (k1);
  - the GEMM sweeps chunk-major per BLK_ORDER so the prefix/copy work
    amortizes over NGRP blocks, with the first chunks' last vocab group
    deferred to the sweep tail so its weights stay off the startup bus;
  - gathers, weights, and constants are ordered/gated so the DMA-bus FIFO
    serves each consumer just before PE needs it (see GATE_A/GATE_B);
  - the globally-last store group goes out per-tile so the tail transfer
    starts right after each final eviction.

Raw Bass with manual semaphores (one wait per instruction): the walrus build
in this container rejects instructions carrying multiple sem waits, which
rules out the Tile scheduler's multi-wait output.

DMA semaphore discipline: a DMA's 16 per-SDMA-engine sem increments interleave
arbitrarily with other in-flight DMAs on the same semaphore, so a summed
threshold across several outstanding DMAs can fire before a lagging engine
lands its data.  Every concurrently-outstanding DMA group therefore gets its
own semaphore, waited to exactly 16 per iteration.

reps>1 repeats the whole pipeline inside one NEFF (used only for timing: the
benchmark fits a line over reps to cancel the ~50-100 ms axon dispatch
overhead).  Iterations re-gather from the table so every rep computes
identical values; cross-iteration WAR hazards get explicit waits.
"""

import functools
import os
from contextlib import ExitStack

import ml_dtypes
import numpy as np

import concourse.bass as bass
from concourse import mybir
from concourse.bass_utils import run_bass_kernel_spmd

B, T, V, D = 8, 1024, 8000, 384
P = 128
NCORE = 8
NCHUNK = T // P                 # 8 token chunks per batch
KD = D // P                     # 3 contraction chunks
NV = 500                        # vocab tile (one fp32 PSUM bank)
VGRP = 2                        # vocab tiles per store/weight group
NSTAGE = 8                      # output staging buffers
GEMM_BANKS = 5
NCTPS = KD                      # one full psum bank per k chain (slot = k);
                                # psum matmul targets must be bank-aligned
F32 = mybir.dt.float32
F32R = mybir.dt.float32r
BF16 = mybir.dt.bfloat16

NVG = int(os.environ.get("BOW_NVG", "2"))   # vocab groups (1 or 2)
NB = NVG                        # batches per core (B=8, 8 cores)
V_CORE = V // NVG               # vocab columns per core
BT = NB * T                     # tokens per core
NCHT = NB * NCHUNK              # token chunks per core
NTV = V_CORE // NV              # vocab tiles per core
NGRP = NTV // VGRP              # store/weight column groups
GCOLS = VGRP * NV               # columns per weight/store group

# gathers: one single-chunk indirect DMA each.  NOTE: batched multi-column
# offset APs simulate correctly in CoreSim but walrus lowers them to a
# DIFFERENT element order on the wire -- compiled results come back wrong.
# Keep one offset column per indirect DMA.
GG = [(c, c + 1) for c in range(NCHT)]
NGG = len(GG)
GRP_OF_CHUNK = {}
for _gi, (_a, _b) in enumerate(GG):
    for _c in range(_a, _b):
        GRP_OF_CHUNK[_c] = _gi

# const-block column layout (single DMA, single sem)
C_IDENC = 0                     # [128, 8]   1/den[c*128+p] column layout
C_DENROW = C_IDENC + NCHUNK     # row 0, [1, 1024] den[t] (bias path only)
C_BIAS = C_DENROW + T           # row 0, [1, V_CORE] (only when has_bias)
CW_NOBIAS = C_IDENC + NCHUNK
CW_BIAS = C_BIAS + V_CORE

# per-iteration semaphore increments
CT_IT = NCHT * KD               # ctdone (prefix matmuls)
GM_IT = NCHT * NTV              # pegemm (gemm psum tiles)
GR_IT = GM_IT // VGRP           # store groups

# GEMM block production order: chunk-major (all vocab groups of one token
# chunk per step) so the prefix/copy-chain work amortizes over NGRP blocks
# instead of landing on every block -- EXCEPT the last vocab group of the
# first 3 chunks, which is deferred to the very end of the sweep: the DMA
# bus cannot deliver group 3's weights before the plain chunk-major order
# reaches them, and their token chunks' NT columns stay resident anyway.
_DEFER = [(mc, NGRP - 1) for mc in range(min(int(os.environ.get("BOW_DEFER", "3")), NCHT))]
_DEFER += [(mc, NGRP - 2) for mc in range(min(int(os.environ.get("BOW_DEFER2", "0")), NCHT))]
BLK_ORDER = [(mc, g) for mc in range(NCHT) for g in range(NGRP)
             if (mc, g) not in _DEFER] + _DEFER
TILE_MC = [mc for (mc, g) in BLK_ORDER for _ in range(VGRP)]
FIRST_POS_OF_G = {}
for _pos, (_mc, _g) in enumerate(BLK_ORDER):
    FIRST_POS_OF_G.setdefault(_g, _pos)

# gather-gate positions: the chunk at GATE_A waits for the group-2 weights,
# the chunk at GATE_B for the group-3 weights (DMA-bus priority, see above)
GATE_A = int(os.environ.get("BOW_GATE_A", "3"))
GATE_B = int(os.environ.get("BOW_GATE_B", "5"))

# eviction -> engine map: alternate DVE/ACT (GPSIMD cannot access PSUM --
# walrus birverifier rejects it -- so Pool only runs the gathers)
EVP = [os.environ.get("BOW_EVP", "DA")[i % 2] for i in range(GM_IT)]
EV_CNT = {e: [0] * (GM_IT + 1) for e in "DA"}
for _i, _e in enumerate(EVP):
    for _en in "DA":
        EV_CNT[_en][_i + 1] = EV_CNT[_en][_i] + (_e == _en)
EV_TOT = {e: EV_CNT[e][GM_IT] for e in "DA"}


def _build(has_bias: bool, gemm_f32r: bool, reps: int = 1):
    nc = bass.Bass("TRN2", target_bir_lowering=False, debug=False)

    mmdt = BF16                 # GEMM operands: W and the NT chain in bf16
    CW = CW_BIAS if has_bias else CW_NOBIAS

    idx_d = nc.dram_tensor("idx", [P, NCHT], mybir.dt.int32, kind="ExternalInput")
    table_d = nc.dram_tensor("table", [V, D], BF16, kind="ExternalInput")
    wt_d = nc.dram_tensor("wt", [D, V_CORE], mmdt, kind="ExternalInput")
    uth_d = nc.dram_tensor("uth", [P, NCHUNK * P], BF16, kind="ExternalInput")
    consts_d = nc.dram_tensor("consts", [P, CW], F32, kind="ExternalInput")
    out_d = nc.dram_tensor("out", [BT, V_CORE], BF16, kind="ExternalOutput")

    with ExitStack() as ctx:
        e = ctx.enter_context
        # SBUF
        idx_sb = e(nc.sbuf_tensor("idx_sb", [P, NCHT], mybir.dt.int32))
        cst = e(nc.sbuf_tensor("cst", [P, CW], F32))
        uth_sb = e(nc.sbuf_tensor("uth_sb", [P, NCHUNK * P], BF16))
        emb_sb = e(nc.sbuf_tensor("emb_sb", [P, NCHT * D], BF16))
        ct_sb = [e(nc.sbuf_tensor(f"ct{k}", [P, BT], mmdt)) for k in range(KD)]
        wt_sb = [e(nc.sbuf_tensor(f"wt{k}", [P, V_CORE], mmdt)) for k in range(KD)]
        ostg = [e(nc.sbuf_tensor(f"ostg{q}", [P, GCOLS], BF16)) for q in range(NSTAGE)]
        carry_sb = e(nc.sbuf_tensor("carry_sb", [P, KD], F32))
        # PSUM (8 banks: 6 gemm + 2 prefix, 2 slots each)
        gps = [e(nc.psum_tensor(f"gps{i}", [P, NV], F32)) for i in range(GEMM_BANKS)]
        ctps_t = [e(nc.psum_tensor(f"ctps{i}", [P, P], F32)) for i in range(NCTPS)]
        ctps = lambda j: ctps_t[j % NCTPS][:]
        # sems -- one per concurrently-outstanding DMA group
        csem = e(nc.semaphore("csem"))
        usem = e(nc.semaphore("usem"))
        wsem = [[e(nc.semaphore(f"wsem{k}_{g}")) for g in range(NGRP)] for k in range(KD)]
        gidx = e(nc.semaphore("gidx"))
        gsem = [e(nc.semaphore(f"gsem{gg}")) for gg in range(NGG)]
        osem = [e(nc.semaphore(f"osem{q}")) for q in range(NSTAGE)]
        # engine-progress sems (single-inc, exactly ordered)
        ctdone = e(nc.semaphore("ctdone"))  # prefix psum tiles done
        ctsb = [e(nc.semaphore(f"ctsb{k}")) for k in range(KD)]  # NT copies
        pegemm = e(nc.semaphore("pegemm"))  # gemm psum tiles done
        evsem = {en: e(nc.semaphore(f"ev{en}")) for en in "DA"}
        blk = e(nc.Block())

        idenc_ap = lambda c: cst[:, C_IDENC + c:C_IDENC + c + 1]
        denrow_ap = lambda c: cst[0:1, C_DENROW + c * P:C_DENROW + (c + 1) * P]
        bias_ap = lambda n: cst[0:1, C_BIAS + n * NV:C_BIAS + (n + 1) * NV]
        utw_ap = lambda c: uth_sb[:, c * P:(c + 1) * P]

        def ev_wait(eng, i):
            # wait for gemm-psum eviction i (global) to be complete
            it, i0 = divmod(i, GM_IT)
            en = EVP[i0]
            eng.wait_ge(evsem[en], it * EV_TOT[en] + EV_CNT[en][i0 + 1])

        @blk.sync
        def _(sync):
            sync.dma_start(idx_sb[:], idx_d[:]).then_inc(gidx, 16)
            # weights in sweep-demand order; group 3 is deferred in
            # BLK_ORDER so its DMAs can sit last.  cst (needed only by the
            # first eviction, ~9us) goes after group 1 to keep its HWDGE
            # descriptor-gen slot off the weight critical path.
            def weights(g):
                for k in range(KD):
                    sync.dma_start(
                        wt_sb[k][:, g * GCOLS:(g + 1) * GCOLS],
                        wt_d[k * P:(k + 1) * P, g * GCOLS:(g + 1) * GCOLS],
                    ).then_inc(wsem[k][g], 16)
            weights(0)
            # group 1+ waits for uth: their bus arrivals then land just
            # behind the chunk-0 gather instead of in front of it
            sync.wait_ge(usem, 16)
            for g in range(1, NGRP):
                weights(g)
            # output stores (the globally-last group goes out as per-tile
            # stores so the tail transfer starts right after each eviction)
            slot_cnt = [0] * NSTAGE
            for it in range(reps):
                for pos, (mc, g) in enumerate(BLK_ORDER):
                    gi = it * GR_IT + pos
                    i0_lo = pos * VGRP
                    slot = gi % NSTAGE
                    rows = out_d[mc * P:(mc + 1) * P, :]
                    if gi == reps * GR_IT - 1:
                        for nin in range(VGRP):
                            i0 = i0_lo + nin
                            en = EVP[i0]
                            sync.wait_ge(evsem[en],
                                         it * EV_TOT[en] + EV_CNT[en][i0 + 1])
                            col = g * GCOLS + nin * NV
                            sync.dma_start(
                                rows[:, col:col + NV],
                                ostg[slot][:, nin * NV:(nin + 1) * NV],
                            ).then_inc(osem[slot], 16)
                            slot_cnt[slot] += 1
                        continue
                    for en in "DA":
                        lo = EV_CNT[en][i0_lo]
                        hi = EV_CNT[en][i0_lo + VGRP]
                        if hi > lo:
                            sync.wait_ge(evsem[en], it * EV_TOT[en] + hi)
                    sync.dma_start(
                        rows[:, g * GCOLS:(g + 1) * GCOLS],
                        ostg[slot][:],
                    ).then_inc(osem[slot], 16)
                    slot_cnt[slot] += 1
            for q in range(NSTAGE):
                if slot_cnt[q]:
                    sync.wait_ge(osem[q], 16 * slot_cnt[q])

        def eviction(eng, en, i0, it, state):
            # evict gemm psum tile i0 (of iteration it) on engine en
            if not state[0]:
                eng.wait_ge(csem, 16)           # idenc scalars
                state[0] = True
            i = it * GM_IT + i0
            gi = i // VGRP
            mc = TILE_MC[i0]
            if EV_CNT[en][i0] == EV_CNT[en][(i0 // VGRP) * VGRP] and gi >= NSTAGE:
                # first touch of staging slot gi%NSTAGE by this engine
                eng.wait_ge(osem[gi % NSTAGE], 16 * (gi // NSTAGE))
            eng.wait_ge(pegemm, i + 1)
            dst = ostg[gi % NSTAGE][:, (i0 % VGRP) * NV:(i0 % VGRP + 1) * NV]
            src, scl = gps[i % GEMM_BANKS][:], idenc_ap(mc % NCHUNK)
            if en == "A":
                op = eng.mul(dst, src, scl)         # ACT: out = in * scale
            else:
                op = eng.tensor_scalar_mul(dst, src, scl)
            op.then_inc(evsem[en], 1)

        def ev_range(eng, en, lo, hi, it, state):
            for i0 in range(lo, hi):
                if EVP[i0] == en:
                    eviction(eng, en, i0, it, state)

        def copy_ct(eng, k, cc, it, inorder, carry_upd):
            # NT carry-chain step for d-slice k of chunk cc: (1) the chunk
            # copy psum+carry -> ct_sb (bf16), (2) a tiny f32 carry update
            # carry_sb[:,k] (+)= psum[:,last] so the running total never
            # loses precision to bf16.  Each op bumps ctsb[k] (2 per chunk).
            # The k chains are split over ACT/DVE so a chunk's copies
            # overlap; ACT's FIFO orders its chains for free, DVE's exec
            # queue reorders so its chain is sem-synced explicitly.
            j = it * CT_IT + cc * KD + k
            base = it * 2 * NCHT
            fresh = cc % NCHUNK == 0        # new batch: no carry to add
            dst = ct_sb[k][:, cc * P:(cc + 1) * P]
            carry = carry_sb[:, k:k + 1]
            last_col = ctps(j)[:, P - 1:P]
            if not carry_upd:
                eng.wait_ge(ctdone, j + 1)
                if it > 0 and cc == 0:
                    # WAR: gemm of iter it-1 must be done reading ct_sb
                    eng.wait_ge(pegemm, it * GM_IT)
                if fresh:
                    if inorder:
                        op = eng.copy(dst, ctps(j))
                    else:
                        op = eng.tensor_scalar_add(dst, ctps(j), 0.0)
                else:
                    if not inorder:
                        eng.wait_ge(ctsb[k], base + 2 * cc)
                    if inorder:
                        op = eng.add(dst, ctps(j), carry)
                    else:
                        op = eng.tensor_scalar_add(dst, ctps(j), carry)
            else:
                if not inorder:
                    eng.wait_ge(ctsb[k], base + 2 * cc + 1)
                if fresh:
                    if inorder:
                        op = eng.copy(carry, last_col)
                    else:
                        op = eng.tensor_scalar_add(carry, last_col, 0.0)
                else:
                    if inorder:
                        op = eng.add(carry, last_col, carry)
                    else:
                        op = eng.tensor_scalar_add(carry, last_col, carry)
            op.then_inc(ctsb[k], 1)

        def chain(eng, ks, en, it, state, inorder):
            # interleave this engine's copy chains with its evictions in
            # PE-production order (prefix mc+1 lands after block (mc, g=1)).
            # All chunk copies go before all carry updates: the GEMM waits
            # only on the copies, the carries have a full chunk-step of
            # slack.
            def copies(cc):
                for k in ks:
                    copy_ct(eng, k, cc, it, inorder, carry_upd=False)
                for k in ks:
                    copy_ct(eng, k, cc, it, inorder, carry_upd=True)

            copies(0)
            for pos, (mc, g) in enumerate(BLK_ORDER):
                ev_range(eng, en, pos * VGRP, (pos + 1) * VGRP, it, state)
                if g == 0 and mc + 1 < NCHT:
                    copies(mc + 1)

        @blk.gpsimd
        def _(gpsimd):
            # uth and cst ride Pool's SWDGE: their descgen fits entirely
            # inside the idx-load latency, so they cost no HWDGE slot and
            # uth (which gates the first prefix) hits the bus first
            gpsimd.dma_start(uth_sb[:], uth_d[:]).then_inc(usem, 16)
            gpsimd.dma_start(cst[:], consts_d[:]).then_inc(csem, 16)
            gpsimd.wait_ge(gidx, 16)
            for it in range(reps):
                for gg, (a, b) in enumerate(GG):
                    if it == 0 and a == 3:
                        # later chunks aren't needed until deep into the
                        # sweep; keep their transfers out of the DMA-bus
                        # queue ahead of the group-2/3 weights that gate
                        # the 3rd/4th GEMM blocks
                        gpsimd.wait_ge(wsem[KD - 1][min(2, NGRP - 1)], 16)
                    if it == 0 and a == 5:
                        gpsimd.wait_ge(wsem[KD - 1][NGRP - 1], 16)
                    if it > 0:
                        # WAR: PE must be done reading these emb chunks (it-1)
                        gpsimd.wait_ge(ctdone, (it - 1) * CT_IT + b * KD)
                    gpsimd.indirect_dma_start(
                        out=emb_sb[:, a * D:b * D],
                        out_offset=None,
                        in_=table_d[:],
                        in_offset=bass.IndirectOffsetOnAxis(ap=idx_sb[:, a:b], axis=0),
                    ).then_inc(gsem[gg], 16)
        @blk.scalar
        def _(scalar):
            st = [False]
            for it in range(reps):
                chain(scalar, (0, 2), "A", it, st, inorder=True)

        @blk.tensor
        def _(tensor):
            tensor.wait_ge(usem, 16)
            if has_bias:
                tensor.wait_ge(csem, 16)
            for it in range(reps):
                def prefix(cc):
                    # one matmul per (chunk, d-slice); the carry is applied
                    # later by the copy chain, not here
                    tensor.wait_ge(gsem[GRP_OF_CHUNK[cc]], 16 * (it + 1))
                    for k in range(KD):
                        j = it * CT_IT + cc * KD + k
                        if j >= NCTPS:
                            # WAR on the ctps slot: both chain ops of the
                            # previous chunk's same-k slice must be done
                            jj = it * CT_IT + cc * KD + k - NCTPS
                            it2, r = divmod(jj, CT_IT)
                            tensor.wait_ge(ctsb[r % KD],
                                           it2 * 2 * NCHT + 2 * (r // KD) + 2)
                        tensor.matmul(
                            ctps(j),
                            lhsT=emb_sb[:, cc * D + k * P: cc * D + (k + 1) * P],
                            rhs=utw_ap(cc % NCHUNK),
                            start=True, stop=True).then_inc(ctdone, 1)

                def gemm_block(pos):
                    mc, g = BLK_ORDER[pos]
                    if it == 0 and pos == FIRST_POS_OF_G[g]:
                        # weight waits first: usually satisfied long before
                        # the ctsb waits below, so the in-order SEQ doesn't
                        # pay their processing after the blocking wait
                        for k in range(KD):
                            tensor.wait_ge(wsem[k][g], 16)
                    if g == 0:
                        for k in range(KD):
                            tensor.wait_ge(ctsb[k], it * 2 * NCHT + 2 * mc + 1)
                    for nin in range(VGRP):
                        n = g * VGRP + nin
                        i = it * GM_IT + pos * VGRP + nin
                        if i >= GEMM_BANKS:
                            ev_wait(tensor, i - GEMM_BANKS)
                        for k in range(KD):
                            last = (k == KD - 1) and not has_bias
                            mm = tensor.matmul(
                                gps[i % GEMM_BANKS][:],
                                lhsT=ct_sb[k][:, mc * P:(mc + 1) * P],
                                rhs=wt_sb[k][:, n * NV:(n + 1) * NV],
                                start=(k == 0), stop=last)
                        if has_bias:
                            mm = tensor.matmul(
                                gps[i % GEMM_BANKS][:],
                                lhsT=denrow_ap(mc % NCHUNK),
                                rhs=bias_ap(n),
                                start=False, stop=True)
                        mm.then_inc(pegemm, 1)

                # chunk-major sweep per BLK_ORDER; prefixes are emitted
                # just-in-time after each chunk's g==1 block (>=1 chunk-step
                # of lead over their first consumer)
                prefix(0)
                for pos, (mc, g) in enumerate(BLK_ORDER):
                    gemm_block(pos)
                    if g == 0 and mc + 1 < NCHT:
                        prefix(mc + 1)

        @blk.vector
        def _(vector):
            st = [False]
            for it in range(reps):
                chain(vector, (1,), "D", it, st, inorder=False)

    return nc


@functools.lru_cache(maxsize=None)
def _get_program(has_bias: bool, gemm_f32r: bool, reps: int = 1):
    return _build(has_bias, gemm_f32r, reps)


@functools.lru_cache(maxsize=None)
def _host_consts(has_bias: bool):
    CW = CW_BIAS if has_bias else CW_NOBIAS
    cst = np.zeros((P, CW), dtype=np.float32)
    t = np.arange(T, dtype=np.float64)
    den = (t + 1.0) * (t + 2.0) / 2.0
    invden = (1.0 / den).astype(np.float32)
    cst[:, C_IDENC:C_IDENC + NCHUNK] = invden.reshape(NCHUNK, P).T
    if has_bias:
        cst[0, C_DENROW:C_DENROW + T] = den.astype(np.float32)
    return cst


@functools.lru_cache(maxsize=None)
def _host_uth():
    # UTW_c[s, t] = (c*128 + s + 1) * [s <= t], bf16, c-major blocks
    s = np.arange(P)
    ut = (s[:, None] <= s[None, :]).astype(np.float32)
    uth = np.empty((P, NCHUNK * P), dtype=ml_dtypes.bfloat16)
    for c in range(NCHUNK):
        w = (c * P + s + 1).astype(np.float32)
        uth[:, c * P:(c + 1) * P] = (w[:, None] * ut).astype(ml_dtypes.bfloat16)
    return uth


GEMM_F32R = os.environ.get("BOW_F32R", "1") == "1"  # fp32r: 4x fp32 PE throughput


def make_in_maps(context, emb_table, W, b):
    context = np.asarray(context)
    emb_table = np.ascontiguousarray(
        np.asarray(emb_table, dtype=np.float32).astype(ml_dtypes.bfloat16))
    W = np.asarray(W, dtype=np.float32)
    b = np.asarray(b, dtype=np.float32)
    has_bias = bool(np.any(b))

    wt_full = np.ascontiguousarray(W.T)  # (D, V)
    cst0 = _host_consts(has_bias)
    uth = _host_uth()

    in_maps = []
    for ci in range(NCORE):
        vg, bg = ci % NVG, ci // NVG
        idx = np.concatenate(
            [context[bg * NB + bt].reshape(NCHUNK, P).T for bt in range(NB)],
            axis=1).astype(np.int32)           # [p, cc]
        wt = np.ascontiguousarray(
            wt_full[:, vg * V_CORE:(vg + 1) * V_CORE].astype(ml_dtypes.bfloat16))
        cst = cst0
        if has_bias:
            cst = cst0.copy()
            cst[0, C_BIAS:C_BIAS + V_CORE] = b[vg * V_CORE:(vg + 1) * V_CORE]
        in_maps.append({"idx": np.ascontiguousarray(idx), "table": emb_table,
                        "wt": wt, "uth": uth, "consts": cst})
    return in_maps, has_bias


def kernel(context, emb_table, W, b):
    in_maps, has_bias = make_in_maps(context, emb_table, W, b)
    nc = _get_program(has_bias, GEMM_F32R)
    try:
        res = run_bass_kernel_spmd(nc, in_maps, list(range(NCORE)))
    except Exception:
        # the axon-tunneled device occasionally reports a transient
        # NRT_EXEC_UNIT_UNRECOVERABLE; one retry reliably clears it
        import time
        time.sleep(2.0)
        res = run_bass_kernel_spmd(nc, in_maps, list(range(NCORE)))
    out = np.empty((B, T, V), dtype=np.float32)
    for ci in range(NCORE):
        vg, bg = ci % NVG, ci // NVG
        o = np.asarray(res.results[ci]["out"]).astype(np.float32)
        for bt in range(NB):
            out[bg * NB + bt, :, vg * V_CORE:(vg + 1) * V_CORE] = \
                o[bt * T:(bt + 1) * T]
    return out
